# revision 21
# baseline (speedup 1.0000x reference)
"""nn_MultiHeadAttention on 8 TRN2 NeuronCores.

IMPORTANT semantics: the reference does a RAW reshape (torch .view style)
  k.reshape(B*H, Lk, d)   with k = [B, L, D] and H*d = D, L = 2048, d = 64.
Since L*D = H * (L*d), "head" i = (b, h) is NOT a feature slice: it is the
contiguous slab = tokens [h*128, (h+1)*128) of batch b, with those 128
tokens' full 1024-dim projected features re-chunked into 2048 virtual
positions m = tok_local*16 + cc, each of 64 dims (cc = feature chunk).

Sharding: token/head-parallel. Core c owns heads {2c, 2c+1} x both batches
= 4 blocks of 128 tokens (bh-major order, bh = b*2 + hl). Everything is
local to the core: projections for its 512 tokens (full D columns),
attention for its 4 heads, and its 512 rows of the output projection.
Host just concatenates token slices (and unscrambles the probs layout).

Device layout per core:
  xT      [1025, 512]  x^T slice + ones row (projection bias via matmul)
  qn/kn   8 tiles [128(col), 512(tok)] f32r  - transposed projections
  qs      8 tiles - qn with chunks shifted one position (SBUF->SBUF DMA)
           so any (cck, ccq) parity pair can share array row groups
  vn -> vaug per bh: [128(tok), 16*65] bf16: per chunk 64 v-cols + ones col
  scores^T blocks [tk=128, tq=128] via K=64 matmuls, 4 per PSUM bank
  softmax denominator = ones-column row of the PV matmul (row 64 of PSUM)
  probs written as [bh, g, cck, tk, (ccq_l, tq)] - host permutes
  o-proj: ctx^T tiles [128(col), 128(tok)] (placed via SBUF->SBUF DMA)
          @ Wo tiles, bias row via ones matmul. opart [512, 1024] local.
"""

import sys

_REPO = "/opt/trn_rl_repo"
if _REPO not in sys.path:
    sys.path.insert(0, _REPO)

import numpy as np
import ml_dtypes

# ---------------------------------------------------------------- config
EXPS_BF16 = True      # exp(scores)/probs-out/v_aug in bf16 (else f32r)
IN_BF16 = True       # activations/weights bf16 (else f32 + f32r matmuls)

B, L, D = 2, 2048, 1024
H, DH = 16, 64
HPC = 2                       # heads per core
N_CORES = 8
TOKC = 512                    # tokens per core (4 bh-blocks of 128)
NBH = B * HPC                 # 4 local (batch, head) units
P = 128
FT = 8                        # full 128-row feature tiles (+1 bias row)
NCC = 16                      # feature chunks of 64 (virtual positions)

_prog = {}


def _build():
    import concourse.bass as bass
    import concourse.bacc as bacc
    import concourse.mybir as mybir
    import concourse.tile as tile

    F32 = mybir.dt.float32
    F32R = mybir.dt.float32r
    BF16 = mybir.dt.bfloat16
    Exp = mybir.ActivationFunctionType.Exp
    MULT = mybir.AluOpType.mult
    PSUM = bass.MemorySpace.PSUM

    IN_DT = BF16 if IN_BF16 else F32R
    EX_DT = BF16 if EXPS_BF16 else F32R

    nc = bacc.Bacc("TRN2", target_bir_lowering=False, debug=False,
                   num_devices=N_CORES)

    xq = nc.declare_dram_parameter("xq", [D + 1, TOKC], IN_DT, isOutput=False)
    xk = nc.declare_dram_parameter("xk", [D + 1, TOKC], IN_DT, isOutput=False)
    xv = nc.declare_dram_parameter("xv", [D + 1, TOKC], IN_DT, isOutput=False)
    wq = nc.declare_dram_parameter("wq", [D + 1, D], IN_DT, isOutput=False)
    wk = nc.declare_dram_parameter("wk", [D + 1, D], IN_DT, isOutput=False)
    wv = nc.declare_dram_parameter("wv", [D + 1, D], IN_DT, isOutput=False)
    wo = nc.declare_dram_parameter("wo", [D + 1, D], IN_DT, isOutput=False)
    probs = nc.declare_dram_parameter("probs", [NBH, 4, P, NCC * 512], EX_DT,
                                      isOutput=True)
    opart = nc.declare_dram_parameter("opart", [TOKC, D], F32, isOutput=True)

    with tile.TileContext(nc) as tc:
        with (
            tc.tile_pool(name="qkst", bufs=1) as qkst,
            tc.tile_pool(name="vaugp", bufs=1) as vaugp,
            tc.tile_pool(name="ctxtp", bufs=1) as ctxtp,
            tc.tile_pool(name="wop", bufs=1) as wop,
            tc.tile_pool(name="onesp", bufs=1) as onesp,
        ):
            kn = [qkst.tile([P, TOKC], IN_DT, tag=f"kn{j}", name=f"kn{j}")
                  for j in range(8)]
            # per-bh q^T in head-sequence order [dd, mq=(ccq,tq)],
            # duplicated to both partition halves so it can pair with
            # either row-group parity of the k chunks
            q2 = [qkst.tile([P, L], IN_DT, tag=f"q2{bh}", name=f"q2{bh}")
                  for bh in range(NBH)]
            vaug = [vaugp.tile([P, NCC * (DH + 1)], EX_DT, tag=f"va{bh}",
                               name=f"va{bh}") for bh in range(NBH)]
            ctxT = [ctxtp.tile([P, 8 * P], IN_DT, tag=f"cx{bh}",
                               name=f"cx{bh}") for bh in range(NBH)]
            wo_t = [[wop.tile([P, 512], IN_DT, tag=f"wo{j}_{oc}",
                              name=f"wo{j}_{oc}") for oc in range(2)]
                    for j in range(8)]
            wo_b = wop.tile([1, D], IN_DT, tag="wo_b")
            ones_f = onesp.tile([1, P], F32, tag="ones_f")
            nc.gpsimd.memset(ones_f[:], 1.0)
            ones1 = onesp.tile([1, P], IN_DT, tag="ones1")
            nc.vector.tensor_copy(ones1[:], ones_f[:])

            # ---------------- phase 1: projections
            with (
                tc.tile_pool(name="xsp", bufs=1) as xsp,
                tc.tile_pool(name="wstr", bufs=1) as wstr,
                tc.tile_pool(name="pp1", bufs=6, space=PSUM) as pp1,
            ):
                xt = {}
                xdram = {"q": xq, "k": xk, "v": xv}
                qn = xsp.tile([P, 8 * TOKC], IN_DT, tag="qn", name="qn")

                # q, k in [col, tok] layout; x tiles on the ACT DMA ring,
                # W row-tiles on the SP ring so loads stream in parallel
                for name, dram in (("q", wq), ("k", wk)):
                    wrow = []
                    for ft in range(FT + 1):
                        kp = P if ft < FT else 1
                        t = xsp.tile([kp, TOKC], IN_DT, tag=f"x{name}{ft}",
                                     name=f"x{name}{ft}")
                        nc.scalar.dma_start(t[:],
                                            xdram[name][ft * P:ft * P + kp, :])
                        xt[name, ft] = t
                        wt = wstr.tile([kp, D], IN_DT, tag=f"w{name}{ft}",
                                       name=f"w{name}{ft}")
                        nc.sync.dma_start(wt[:], dram[ft * P:ft * P + kp, :])
                        wrow.append(wt)
                    for ct in range(8):
                        pt = pp1.tile([P, TOKC], F32, tag="pp1", name="pp1")
                        for ft in range(FT + 1):
                            nc.tensor.matmul(
                                pt[:],
                                wrow[ft][:, ct * P:(ct + 1) * P],
                                xt[name, ft][:],
                                start=(ft == 0), stop=(ft == FT))
                        if name == "q":
                            nc.vector.tensor_copy(
                                qn[:, ct * TOKC:(ct + 1) * TOKC], pt[:])
                        else:
                            nc.vector.tensor_copy(kn[ct][:], pt[:])

                # build q2[bh][{0:64,64:128}, ccq*128:+128] = q chunk ccq
                # of this bh's tokens (both halves identical). One strided
                # DMA per (bh, half, parity) gathers all 8 same-parity
                # chunks: src = qn[par*64:+64, ct*512 + bh*128 (+128)],
                # dst free offsets ccq=2*ct+par -> (2*ct+par)*128.
                qnv = qn.rearrange("p (ct t) -> p ct t", t=TOKC)
                for bh in range(NBH):
                    q2v = q2[bh].rearrange("p (ct pr t) -> p ct pr t",
                                           pr=2, t=P)
                    for r in (0, 64):
                        for par in (0, 1):
                            nc.sync.dma_start(
                                q2v[r:r + 64, :, par, :],
                                qnv[par * 64:par * 64 + 64, :,
                                    bh * P:(bh + 1) * P])

                # v in natural [tok, col] layout, straight into vaug (+ones)
                wvrow = []
                for ft in range(FT + 1):
                    kp = P if ft < FT else 1
                    t = xsp.tile([kp, TOKC], IN_DT, tag=f"xv{ft}",
                                 name=f"xv{ft}")
                    nc.scalar.dma_start(t[:], xv[ft * P:ft * P + kp, :])
                    xt["v", ft] = t
                    wt = wstr.tile([kp, D], IN_DT, tag=f"wv{ft}",
                                   name=f"wv{ft}")
                    nc.sync.dma_start(wt[:], wv[ft * P:ft * P + kp, :])
                    wvrow.append(wt)
                for bh in range(NBH):
                    ts = slice(bh * P, (bh + 1) * P)
                    vview = vaug[bh].rearrange("p (c e) -> p c e", e=DH + 1)
                    for oc in range(2):
                        pt = pp1.tile([P, 512], F32, tag="pp1", name="pp1v")
                        for ft in range(FT + 1):
                            nc.tensor.matmul(pt[:], xt["v", ft][:, ts],
                                             wvrow[ft][:, oc * 512:
                                                       oc * 512 + 512],
                                             start=(ft == 0),
                                             stop=(ft == FT))
                        pview = pt.rearrange("p (c e) -> p c e", e=DH)
                        nc.vector.tensor_copy(
                            vview[:, oc * 8:(oc + 1) * 8, 0:DH], pview[:])
                    nc.gpsimd.memset(vview[:, :, DH:DH + 1], 1.0)

            # ---------------- phase 2: attention
            with (
                tc.tile_pool(name="exps", bufs=2) as exps,
                tc.tile_pool(name="bcp", bufs=4) as bcp,
                tc.tile_pool(name="rzp", bufs=4) as rzp,
                tc.tile_pool(name="ctmp", bufs=4) as ctmp,
                tc.tile_pool(name="osb", bufs=4) as osb,
                tc.tile_pool(name="pps", bufs=2, space=PSUM) as pps,
                tc.tile_pool(name="ppav", bufs=3, space=PSUM) as ppav,
                tc.tile_pool(name="ppo", bufs=1, space=PSUM) as ppo,
            ):
                for j in range(8):
                    for oc in range(2):
                        nc.sync.dma_start(
                            wo_t[j][oc][:],
                            wo[j * P:(j + 1) * P, oc * 512:oc * 512 + 512])
                nc.sync.dma_start(wo_b[:], wo[D:D + 1, :])
                for bh in range(NBH):
                    ts = slice(bh * P, (bh + 1) * P)
                    for g in range(4):
                        pav = ppav.tile([DH + 1, 512], F32, tag="pav",
                                        name="pav")
                        eb = exps.tile([P, NCC * 512], EX_DT, tag="exps",
                                       name="exps")
                        for t in range(8):          # cck pairs 2t, 2t+1
                            pss = pps.tile([P, 1024], F32, tag="pss",
                                           name="pss")
                            for par in (0, 1):
                                cck = 2 * t + par
                                rk = par * 64
                                nc.tensor.matmul(
                                    pss[:, par * 512:par * 512 + 512],
                                    kn[cck // 2][rk:rk + 64, ts],
                                    q2[bh][rk:rk + 64,
                                           g * 512:g * 512 + 512])
                            nc.scalar.activation(
                                eb[:, t * 1024:(t + 1) * 1024], pss[:],
                                Exp, scale=0.125)
                            for par in (0, 1):
                                cck = 2 * t + par
                                nc.tensor.matmul(
                                    pav[:],
                                    vaug[bh][:, cck * (DH + 1):
                                             (cck + 1) * (DH + 1)],
                                    eb[:, cck * 512:(cck + 1) * 512],
                                    start=(cck == 0), stop=(cck == 15))
                        # softmax denominators are row 64 of pav; spread
                        # the 512 Z values across all partitions so the
                        # reciprocal runs 128 lanes wide (a [1,512] slice
                        # would be single-lane and ~3.4us)
                        zrow = rzp.tile([P, 512], F32, tag="zrow",
                                        name="zrow")
                        nc.vector.tensor_copy(zrow[64:65, :], pav[64:65, :])
                        zc = rzp.tile([P, 4], F32, tag="zc", name="zc")
                        nc.sync.dma_start(zc[:], zrow[64:65, :])
                        zr = rzp.tile([P, 4], F32, tag="zr", name="zr")
                        nc.vector.reciprocal_approx_fast(zr[:], zc[:])
                        zrb = rzp.tile([P, 4], EX_DT, tag="zrb", name="zrb")
                        nc.vector.tensor_copy(zrb[:], zr[:])
                        rz0 = rzp.tile([1, 512], EX_DT, tag="rz0",
                                       name="rz0")
                        nc.sync.dma_start(rz0[:], zrb[:])
                        bc = bcp.tile([P, 512], EX_DT, tag="bc", name="bc")
                        nc.gpsimd.partition_broadcast(bc[:], rz0[:])
                        for cck in range(NCC):
                            esl = eb[:, cck * 512:(cck + 1) * 512]
                            eng = nc.gpsimd if cck >= 10 else nc.vector
                            eng.tensor_tensor(esl, esl, bc[:], MULT)
                        nc.sync.dma_start(probs[bh, g], eb[:])
                        ct_ = ctmp.tile([DH, 512], IN_DT, tag="ctmp",
                                        name="ctmp")
                        nc.vector.tensor_tensor(ct_[:], pav[0:DH, :],
                                                bc[0:DH, :], MULT)
                        # scatter ct_ chunks into ctxT: ccq=g*4+l ->
                        # tile col-block ccq//2, rows (ccq%2)*64. Group g
                        # covers ct-blocks 2g, 2g+1; parity == l%2.
                        cxv = ctxT[bh].rearrange("p (ct t) -> p ct t", t=P)
                        ctv = ct_.rearrange("p (j pr t) -> p j pr t",
                                            pr=2, t=P)
                        for par in (0, 1):
                            nc.sync.dma_start(
                                cxv[par * 64:par * 64 + 64,
                                    2 * g:2 * g + 2, :],
                                ctv[:, :, par, :])
                    # ---- output projection for this bh block
                    for oc in range(2):
                        po = ppo.tile([P, 512], F32, tag="po", name="po")
                        for j in range(8):
                            nc.tensor.matmul(po[:],
                                             ctxT[bh][:, j * P:(j + 1) * P],
                                             wo_t[j][oc][:],
                                             start=(j == 0), stop=False)
                        nc.tensor.matmul(po[:], ones1[:],
                                         wo_b[:, oc * 512:oc * 512 + 512],
                                         start=False, stop=True)
                        po_sb = osb.tile([P, 512], F32, tag="po_sb",
                                         name="po_sb")
                        if oc == 0:
                            nc.vector.tensor_copy(po_sb[:], po[:])
                        else:
                            nc.scalar.copy(po_sb[:], po[:])
                        nc.sync.dma_start(
                            opart[bh * P:(bh + 1) * P,
                                  oc * 512:oc * 512 + 512], po_sb[:])

    nc.compile()
    return nc


def _get_prog():
    if "nc" not in _prog:
        _prog["nc"] = _build()
    return _prog["nc"]


def kernel(key, value, query, Wk, bk, Wv, bv, Wq, bq, Wo, bo):
    from concourse.bass_utils import run_bass_kernel_spmd

    f32 = np.float32
    in_np = ml_dtypes.bfloat16 if IN_BF16 else f32

    key, value, query = (np.asarray(t, f32) for t in (key, value, query))
    Wk, bk, Wv, bv, Wq, bq, Wo, bo = (
        np.asarray(t, f32) for t in (Wk, bk, Wv, bv, Wq, bq, Wo, bo))

    xf = {"xq": query.reshape(B * L, D), "xk": key.reshape(B * L, D),
          "xv": value.reshape(B * L, D)}
    w_aug = {"wq": np.concatenate([Wq, bq[None, :]], 0).astype(in_np),
             "wk": np.concatenate([Wk, bk[None, :]], 0).astype(in_np),
             "wv": np.concatenate([Wv, bv[None, :]], 0).astype(in_np),
             "wo": np.concatenate([Wo, bo[None, :]], 0).astype(in_np)}

    ones = np.ones((1, TOKC), f32)
    in_maps = []
    for c in range(N_CORES):
        rows = np.concatenate([
            np.arange((bh // HPC) * L + (HPC * c + bh % HPC) * P,
                      (bh // HPC) * L + (HPC * c + bh % HPC) * P + P)
            for bh in range(NBH)])
        m = dict(w_aug)
        for n in ("xq", "xk", "xv"):
            m[n] = np.ascontiguousarray(
                np.concatenate([xf[n][rows].T, ones], 0)).astype(in_np)
        in_maps.append(m)

    nc = _get_prog()
    res = run_bass_kernel_spmd(nc, in_maps, list(range(N_CORES)),
                               trace=False)

    attention = np.empty((B * H, L, L), f32)
    context = np.empty((B, L, D), f32)
    for c in range(N_CORES):
        r = res.results[c]
        # probs [bh, g, tk, (cck, ccq_l, tq)] -> [bh, mq, mk]
        p = np.asarray(r["probs"]).astype(f32)
        p = p.reshape(NBH, 4, P, NCC, 4, P)
        p = p.transpose(0, 5, 1, 4, 2, 3).reshape(NBH, L, L)
        op = np.asarray(r["opart"], f32)
        for bh in range(NBH):
            b, hl = bh // HPC, bh % HPC
            h = HPC * c + hl
            attention[b * H + h] = p[bh]
            context[b, h * P:(h + 1) * P, :] = op[bh * P:(bh + 1) * P, :]
    return context, attention


# revision 22
# speedup vs baseline: 1.3433x; 1.3433x over previous
"""nn_MultiHeadAttention on 8 TRN2 NeuronCores.

IMPORTANT semantics: the reference does a RAW reshape (torch .view style)
  k.reshape(B*H, Lk, d)   with k = [B, L, D] and H*d = D, L = 2048, d = 64.
Since L*D = H * (L*d), "head" i = (b, h) is NOT a feature slice: it is the
contiguous slab = tokens [h*128, (h+1)*128) of batch b, with those 128
tokens' full 1024-dim projected features re-chunked into 2048 virtual
positions m = tok_local*16 + cc, each of 64 dims (cc = feature chunk).

Sharding: token/head-parallel. Core c owns heads {2c, 2c+1} x both batches
= 4 blocks of 128 tokens (bh-major order, bh = b*2 + hl). Everything is
local to the core: projections for its 512 tokens (full D columns),
attention for its 4 heads, and its 512 rows of the output projection.
Host just concatenates token slices (and unscrambles the probs layout).

Device layout per core:
  xT      [1025, 512]  x^T slice + ones row (projection bias via matmul)
  qn/kn   8 tiles [128(col), 512(tok)] f32r  - transposed projections
  qs      8 tiles - qn with chunks shifted one position (SBUF->SBUF DMA)
           so any (cck, ccq) parity pair can share array row groups
  vn -> vaug per bh: [128(tok), 16*65] bf16: per chunk 64 v-cols + ones col
  scores^T blocks [tk=128, tq=128] via K=64 matmuls, 4 per PSUM bank
  softmax denominator = ones-column row of the PV matmul (row 64 of PSUM)
  probs written as [bh, g, cck, tk, (ccq_l, tq)] - host permutes
  o-proj: ctx^T tiles [128(col), 128(tok)] (placed via SBUF->SBUF DMA)
          @ Wo tiles, bias row via ones matmul. opart [512, 1024] local.
"""

import sys

_REPO = "/opt/trn_rl_repo"
if _REPO not in sys.path:
    sys.path.insert(0, _REPO)

import numpy as np
import ml_dtypes

# ---------------------------------------------------------------- config
EXPS_BF16 = True      # exp(scores)/probs-out/v_aug in bf16 (else f32r)
IN_BF16 = True       # activations/weights bf16 (else f32 + f32r matmuls)

B, L, D = 2, 2048, 1024
H, DH = 16, 64
HPC = 2                       # heads per core
N_CORES = 8
TOKC = 512                    # tokens per core (4 bh-blocks of 128)
NBH = B * HPC                 # 4 local (batch, head) units
P = 128
FT = 8                        # full 128-row feature tiles (+1 bias row)
NCC = 16                      # feature chunks of 64 (virtual positions)

_prog = {}


def _build():
    import concourse.bass as bass
    import concourse.bacc as bacc
    import concourse.mybir as mybir
    import concourse.tile as tile

    F32 = mybir.dt.float32
    F32R = mybir.dt.float32r
    BF16 = mybir.dt.bfloat16
    Exp = mybir.ActivationFunctionType.Exp
    MULT = mybir.AluOpType.mult
    PSUM = bass.MemorySpace.PSUM

    IN_DT = BF16 if IN_BF16 else F32R
    EX_DT = BF16 if EXPS_BF16 else F32R

    nc = bacc.Bacc("TRN2", target_bir_lowering=False, debug=False,
                   num_devices=N_CORES)

    xq = nc.declare_dram_parameter("xq", [D + 1, TOKC], IN_DT, isOutput=False)
    xk = nc.declare_dram_parameter("xk", [D + 1, TOKC], IN_DT, isOutput=False)
    xv = nc.declare_dram_parameter("xv", [D + 1, TOKC], IN_DT, isOutput=False)
    wq = nc.declare_dram_parameter("wq", [D + 1, D], IN_DT, isOutput=False)
    wk = nc.declare_dram_parameter("wk", [D + 1, D], IN_DT, isOutput=False)
    wv = nc.declare_dram_parameter("wv", [D + 1, D], IN_DT, isOutput=False)
    wo = nc.declare_dram_parameter("wo", [D + 1, D], IN_DT, isOutput=False)
    probs = nc.declare_dram_parameter("probs", [NBH, 4, P, NCC * 512], EX_DT,
                                      isOutput=True)
    opart = nc.declare_dram_parameter("opart", [TOKC, D], F32, isOutput=True)

    with tile.TileContext(nc) as tc:
        with (
            tc.tile_pool(name="qkst", bufs=1) as qkst,
            tc.tile_pool(name="vaugp", bufs=1) as vaugp,
            tc.tile_pool(name="ctxtp", bufs=1) as ctxtp,
            tc.tile_pool(name="wop", bufs=1) as wop,
            tc.tile_pool(name="onesp", bufs=1) as onesp,
        ):
            kn = [qkst.tile([P, TOKC], IN_DT, tag=f"kn{j}", name=f"kn{j}")
                  for j in range(8)]
            # per-bh q^T in head-sequence order [dd, mq=(ccq,tq)],
            # duplicated to both partition halves so it can pair with
            # either row-group parity of the k chunks
            q2 = [qkst.tile([P, L], IN_DT, tag=f"q2{bh}", name=f"q2{bh}")
                  for bh in range(NBH)]
            vaug = [vaugp.tile([P, NCC * (DH + 1)], EX_DT, tag=f"va{bh}",
                               name=f"va{bh}") for bh in range(NBH)]
            ctxT = [ctxtp.tile([P, 8 * P], IN_DT, tag=f"cx{bh}",
                               name=f"cx{bh}") for bh in range(NBH)]
            wo_t = [[wop.tile([P, 512], IN_DT, tag=f"wo{j}_{oc}",
                              name=f"wo{j}_{oc}") for oc in range(2)]
                    for j in range(8)]
            wo_b = wop.tile([1, D], IN_DT, tag="wo_b")
            ones_f = onesp.tile([1, P], F32, tag="ones_f")
            nc.gpsimd.memset(ones_f[:], 1.0)
            ones1 = onesp.tile([1, P], IN_DT, tag="ones1")
            nc.vector.tensor_copy(ones1[:], ones_f[:])

            # ---------------- phase 1: projections
            with (
                tc.tile_pool(name="xsp", bufs=1) as xsp,
                tc.tile_pool(name="wstr", bufs=1) as wstr,
                tc.tile_pool(name="pp1", bufs=6, space=PSUM) as pp1,
            ):
                xt = {}
                xdram = {"q": xq, "k": xk, "v": xv}
                qn = xsp.tile([P, 8 * TOKC], IN_DT, tag="qn", name="qn")

                # q, k in [col, tok] layout; x tiles on the ACT DMA ring,
                # W row-tiles on the SP ring so loads stream in parallel
                for name, dram in (("q", wq), ("k", wk)):
                    wrow = []
                    for ft in range(FT + 1):
                        kp = P if ft < FT else 1
                        t = xsp.tile([kp, TOKC], IN_DT, tag=f"x{name}{ft}",
                                     name=f"x{name}{ft}")
                        nc.scalar.dma_start(t[:],
                                            xdram[name][ft * P:ft * P + kp, :])
                        xt[name, ft] = t
                        wt = wstr.tile([kp, D], IN_DT, tag=f"w{name}{ft}",
                                       name=f"w{name}{ft}")
                        nc.sync.dma_start(wt[:], dram[ft * P:ft * P + kp, :])
                        wrow.append(wt)
                    for ct in range(8):
                        pt = pp1.tile([P, TOKC], F32, tag="pp1", name="pp1")
                        for ft in range(FT + 1):
                            nc.tensor.matmul(
                                pt[:],
                                wrow[ft][:, ct * P:(ct + 1) * P],
                                xt[name, ft][:],
                                start=(ft == 0), stop=(ft == FT))
                        if name == "q":
                            nc.vector.tensor_copy(
                                qn[:, ct * TOKC:(ct + 1) * TOKC], pt[:])
                        else:
                            nc.vector.tensor_copy(kn[ct][:], pt[:])

                # build q2[bh][{0:64,64:128}, ccq*128:+128] = q chunk ccq
                # of this bh's tokens (both halves identical). One strided
                # DMA per (bh, half, parity) gathers all 8 same-parity
                # chunks: src = qn[par*64:+64, ct*512 + bh*128 (+128)],
                # dst free offsets ccq=2*ct+par -> (2*ct+par)*128.
                qnv = qn.rearrange("p (ct t) -> p ct t", t=TOKC)
                for bh in range(NBH):
                    q2v = q2[bh].rearrange("p (ct pr t) -> p ct pr t",
                                           pr=2, t=P)
                    for r in (0, 64):
                        for par in (0, 1):
                            nc.sync.dma_start(
                                q2v[r:r + 64, :, par, :],
                                qnv[par * 64:par * 64 + 64, :,
                                    bh * P:(bh + 1) * P])

                # v in natural [tok, col] layout, straight into vaug (+ones)
                wvrow = []
                for ft in range(FT + 1):
                    kp = P if ft < FT else 1
                    t = xsp.tile([kp, TOKC], IN_DT, tag=f"xv{ft}",
                                 name=f"xv{ft}")
                    nc.scalar.dma_start(t[:], xv[ft * P:ft * P + kp, :])
                    xt["v", ft] = t
                    wt = wstr.tile([kp, D], IN_DT, tag=f"wv{ft}",
                                   name=f"wv{ft}")
                    nc.sync.dma_start(wt[:], wv[ft * P:ft * P + kp, :])
                    wvrow.append(wt)
                for bh in range(NBH):
                    ts = slice(bh * P, (bh + 1) * P)
                    vview = vaug[bh].rearrange("p (c e) -> p c e", e=DH + 1)
                    for oc in range(2):
                        pt = pp1.tile([P, 512], F32, tag="pp1", name="pp1v")
                        for ft in range(FT + 1):
                            nc.tensor.matmul(pt[:], xt["v", ft][:, ts],
                                             wvrow[ft][:, oc * 512:
                                                       oc * 512 + 512],
                                             start=(ft == 0),
                                             stop=(ft == FT))
                        pview = pt.rearrange("p (c e) -> p c e", e=DH)
                        nc.vector.tensor_copy(
                            vview[:, oc * 8:(oc + 1) * 8, 0:DH], pview[:])
                    nc.gpsimd.memset(vview[:, :, DH:DH + 1], 1.0)

            # ---------------- phase 2: attention
            with (
                tc.tile_pool(name="exps", bufs=2) as exps,
                tc.tile_pool(name="bcp", bufs=4) as bcp,
                tc.tile_pool(name="rzp", bufs=4) as rzp,
                tc.tile_pool(name="ctmp", bufs=4) as ctmp,
                tc.tile_pool(name="osb", bufs=4) as osb,
                tc.tile_pool(name="pps", bufs=2, space=PSUM) as pps,
                tc.tile_pool(name="ppav", bufs=3, space=PSUM) as ppav,
                tc.tile_pool(name="ppo", bufs=1, space=PSUM) as ppo,
            ):
                for j in range(8):
                    for oc in range(2):
                        nc.sync.dma_start(
                            wo_t[j][oc][:],
                            wo[j * P:(j + 1) * P, oc * 512:oc * 512 + 512])
                nc.sync.dma_start(wo_b[:], wo[D:D + 1, :])
                for bh in range(NBH):
                    ts = slice(bh * P, (bh + 1) * P)
                    for g in range(4):
                        pav = ppav.tile([DH + 1, 512], F32, tag="pav",
                                        name="pav")
                        eb = exps.tile([P, NCC * 512], EX_DT, tag="exps",
                                       name="exps")
                        for t in range(8):          # cck pairs 2t, 2t+1
                            pss = pps.tile([P, 1024], F32, tag="pss",
                                           name="pss")
                            for par in (0, 1):
                                cck = 2 * t + par
                                rk = par * 64
                                nc.tensor.matmul(
                                    pss[:, par * 512:par * 512 + 512],
                                    kn[cck // 2][rk:rk + 64, ts],
                                    q2[bh][rk:rk + 64,
                                           g * 512:g * 512 + 512])
                            nc.scalar.activation(
                                eb[:, t * 1024:(t + 1) * 1024], pss[:],
                                Exp, scale=0.125)
                            for par in (0, 1):
                                cck = 2 * t + par
                                nc.tensor.matmul(
                                    pav[:],
                                    vaug[bh][:, cck * (DH + 1):
                                             (cck + 1) * (DH + 1)],
                                    eb[:, cck * 512:(cck + 1) * 512],
                                    start=(cck == 0), stop=(cck == 15))
                        # softmax denominators are row 64 of pav; spread
                        # the 512 Z values across all partitions so the
                        # reciprocal runs 128 lanes wide (a [1,512] slice
                        # would be single-lane and ~3.4us)
                        zrow = rzp.tile([P, 512], F32, tag="zrow",
                                        name="zrow")
                        nc.vector.tensor_copy(zrow[64:65, :], pav[64:65, :])
                        zc = rzp.tile([P, 4], F32, tag="zc", name="zc")
                        nc.sync.dma_start(zc[:], zrow[64:65, :])
                        zr = rzp.tile([P, 4], F32, tag="zr", name="zr")
                        nc.vector.reciprocal_approx_fast(zr[:], zc[:])
                        zrb = rzp.tile([P, 4], EX_DT, tag="zrb", name="zrb")
                        nc.vector.tensor_copy(zrb[:], zr[:])
                        rz0 = rzp.tile([1, 512], EX_DT, tag="rz0",
                                       name="rz0")
                        nc.sync.dma_start(rz0[:], zrb[:])
                        bc = bcp.tile([P, 512], EX_DT, tag="bc", name="bc")
                        nc.gpsimd.partition_broadcast(bc[:], rz0[:])
                        for cck in range(NCC):
                            esl = eb[:, cck * 512:(cck + 1) * 512]
                            nc.vector.tensor_tensor(esl, esl, bc[:], MULT)
                        nc.sync.dma_start(probs[bh, g], eb[:])
                        ct_ = ctmp.tile([DH, 512], IN_DT, tag="ctmp",
                                        name="ctmp")
                        nc.vector.tensor_tensor(ct_[:], pav[0:DH, :],
                                                bc[0:DH, :], MULT)
                        # scatter ct_ chunks into ctxT: ccq=g*4+l ->
                        # tile col-block ccq//2, rows (ccq%2)*64. Group g
                        # covers ct-blocks 2g, 2g+1; parity == l%2.
                        cxv = ctxT[bh].rearrange("p (ct t) -> p ct t", t=P)
                        ctv = ct_.rearrange("p (j pr t) -> p j pr t",
                                            pr=2, t=P)
                        for par in (0, 1):
                            nc.sync.dma_start(
                                cxv[par * 64:par * 64 + 64,
                                    2 * g:2 * g + 2, :],
                                ctv[:, :, par, :])
                    # ---- output projection for this bh block
                    for oc in range(2):
                        po = ppo.tile([P, 512], F32, tag="po", name="po")
                        for j in range(8):
                            nc.tensor.matmul(po[:],
                                             ctxT[bh][:, j * P:(j + 1) * P],
                                             wo_t[j][oc][:],
                                             start=(j == 0), stop=False)
                        nc.tensor.matmul(po[:], ones1[:],
                                         wo_b[:, oc * 512:oc * 512 + 512],
                                         start=False, stop=True)
                        po_sb = osb.tile([P, 512], F32, tag="po_sb",
                                         name="po_sb")
                        if oc == 0:
                            nc.vector.tensor_copy(po_sb[:], po[:])
                        else:
                            nc.scalar.copy(po_sb[:], po[:])
                        nc.sync.dma_start(
                            opart[bh * P:(bh + 1) * P,
                                  oc * 512:oc * 512 + 512], po_sb[:])

    nc.compile()
    return nc


def _get_prog():
    if "nc" not in _prog:
        _prog["nc"] = _build()
    return _prog["nc"]


def kernel(key, value, query, Wk, bk, Wv, bv, Wq, bq, Wo, bo):
    from concourse.bass_utils import run_bass_kernel_spmd

    f32 = np.float32
    in_np = ml_dtypes.bfloat16 if IN_BF16 else f32

    key, value, query = (np.asarray(t, f32) for t in (key, value, query))
    Wk, bk, Wv, bv, Wq, bq, Wo, bo = (
        np.asarray(t, f32) for t in (Wk, bk, Wv, bv, Wq, bq, Wo, bo))

    xf = {"xq": query.reshape(B * L, D), "xk": key.reshape(B * L, D),
          "xv": value.reshape(B * L, D)}
    w_aug = {"wq": np.concatenate([Wq, bq[None, :]], 0).astype(in_np),
             "wk": np.concatenate([Wk, bk[None, :]], 0).astype(in_np),
             "wv": np.concatenate([Wv, bv[None, :]], 0).astype(in_np),
             "wo": np.concatenate([Wo, bo[None, :]], 0).astype(in_np)}

    ones = np.ones((1, TOKC), f32)
    in_maps = []
    for c in range(N_CORES):
        rows = np.concatenate([
            np.arange((bh // HPC) * L + (HPC * c + bh % HPC) * P,
                      (bh // HPC) * L + (HPC * c + bh % HPC) * P + P)
            for bh in range(NBH)])
        m = dict(w_aug)
        for n in ("xq", "xk", "xv"):
            m[n] = np.ascontiguousarray(
                np.concatenate([xf[n][rows].T, ones], 0)).astype(in_np)
        in_maps.append(m)

    nc = _get_prog()
    res = run_bass_kernel_spmd(nc, in_maps, list(range(N_CORES)),
                               trace=False)

    attention = np.empty((B * H, L, L), f32)
    context = np.empty((B, L, D), f32)
    for c in range(N_CORES):
        r = res.results[c]
        # probs [bh, g, tk, (cck, ccq_l, tq)] -> [bh, mq, mk]
        p = np.asarray(r["probs"]).astype(f32)
        p = p.reshape(NBH, 4, P, NCC, 4, P)
        p = p.transpose(0, 5, 1, 4, 2, 3).reshape(NBH, L, L)
        op = np.asarray(r["opart"], f32)
        for bh in range(NBH):
            b, hl = bh // HPC, bh % HPC
            h = HPC * c + hl
            attention[b * H + h] = p[bh]
            context[b, h * P:(h + 1) * P, :] = op[bh * P:(bh + 1) * P, :]
    return context, attention


# revision 23
# speedup vs baseline: 1.4765x; 1.0992x over previous
"""nn_MultiHeadAttention on 8 TRN2 NeuronCores.

IMPORTANT semantics: the reference does a RAW reshape (torch .view style)
  k.reshape(B*H, Lk, d)   with k = [B, L, D] and H*d = D, L = 2048, d = 64.
Since L*D = H * (L*d), "head" i = (b, h) is NOT a feature slice: it is the
contiguous slab = tokens [h*128, (h+1)*128) of batch b, with those 128
tokens' full 1024-dim projected features re-chunked into 2048 virtual
positions m = tok_local*16 + cc, each of 64 dims (cc = feature chunk).

Sharding: token/head-parallel. Core c owns heads {2c, 2c+1} x both batches
= 4 blocks of 128 tokens (bh-major order, bh = b*2 + hl). Everything is
local to the core: projections for its 512 tokens (full D columns),
attention for its 4 heads, and its 512 rows of the output projection.
Host just concatenates token slices (and unscrambles the probs layout).

Device layout per core:
  xT      [1025, 512]  x^T slice + ones row (projection bias via matmul)
  qn/kn   8 tiles [128(col), 512(tok)] f32r  - transposed projections
  qs      8 tiles - qn with chunks shifted one position (SBUF->SBUF DMA)
           so any (cck, ccq) parity pair can share array row groups
  vn -> vaug per bh: [128(tok), 16*65] bf16: per chunk 64 v-cols + ones col
  scores^T blocks [tk=128, tq=128] via K=64 matmuls, 4 per PSUM bank
  softmax denominator = ones-column row of the PV matmul (row 64 of PSUM)
  probs written as [bh, g, cck, tk, (ccq_l, tq)] - host permutes
  o-proj: ctx^T tiles [128(col), 128(tok)] (placed via SBUF->SBUF DMA)
          @ Wo tiles, bias row via ones matmul. opart [512, 1024] local.
"""

import sys

_REPO = "/opt/trn_rl_repo"
if _REPO not in sys.path:
    sys.path.insert(0, _REPO)

import numpy as np
import ml_dtypes

# ---------------------------------------------------------------- config
EXPS_BF16 = True      # exp(scores)/probs-out/v_aug in bf16 (else f32r)
IN_BF16 = True       # activations/weights bf16 (else f32 + f32r matmuls)

B, L, D = 2, 2048, 1024
H, DH = 16, 64
HPC = 2                       # heads per core
N_CORES = 8
TOKC = 512                    # tokens per core (4 bh-blocks of 128)
NBH = B * HPC                 # 4 local (batch, head) units
P = 128
FT = 8                        # full 128-row feature tiles (+1 bias row)
NCC = 16                      # feature chunks of 64 (virtual positions)

_prog = {}


def _build():
    import concourse.bass as bass
    import concourse.bacc as bacc
    import concourse.mybir as mybir
    import concourse.tile as tile

    F32 = mybir.dt.float32
    F32R = mybir.dt.float32r
    BF16 = mybir.dt.bfloat16
    Exp = mybir.ActivationFunctionType.Exp
    MULT = mybir.AluOpType.mult
    PSUM = bass.MemorySpace.PSUM

    IN_DT = BF16 if IN_BF16 else F32R
    EX_DT = BF16 if EXPS_BF16 else F32R

    nc = bacc.Bacc("TRN2", target_bir_lowering=False, debug=False,
                   num_devices=N_CORES)

    xq = nc.declare_dram_parameter("xq", [D + 1, TOKC], IN_DT, isOutput=False)
    xk = nc.declare_dram_parameter("xk", [D + 1, TOKC], IN_DT, isOutput=False)
    xv = nc.declare_dram_parameter("xv", [D + 1, TOKC], IN_DT, isOutput=False)
    wq = nc.declare_dram_parameter("wq", [D + 1, D], IN_DT, isOutput=False)
    wk = nc.declare_dram_parameter("wk", [D + 1, D], IN_DT, isOutput=False)
    wv = nc.declare_dram_parameter("wv", [D + 1, D], IN_DT, isOutput=False)
    wo = nc.declare_dram_parameter("wo", [D + 1, D], IN_DT, isOutput=False)
    probs = nc.declare_dram_parameter("probs", [NBH, 4, P, NCC * 512], EX_DT,
                                      isOutput=True)
    opart = nc.declare_dram_parameter("opart", [TOKC, D], F32, isOutput=True)

    with tile.TileContext(nc) as tc:
        with (
            tc.tile_pool(name="qkst", bufs=1) as qkst,
            tc.tile_pool(name="vaugp", bufs=1) as vaugp,
            tc.tile_pool(name="ctxtp", bufs=1) as ctxtp,
            tc.tile_pool(name="wop", bufs=1) as wop,
            tc.tile_pool(name="onesp", bufs=1) as onesp,
        ):
            kn = [qkst.tile([P, TOKC], IN_DT, tag=f"kn{j}", name=f"kn{j}")
                  for j in range(8)]
            # per-bh q^T in head-sequence order [dd, mq=(ccq,tq)],
            # duplicated to both partition halves so it can pair with
            # either row-group parity of the k chunks
            q2 = [qkst.tile([P, L], IN_DT, tag=f"q2{bh}", name=f"q2{bh}")
                  for bh in range(NBH)]
            vaug = [vaugp.tile([P, NCC * (DH + 1)], EX_DT, tag=f"va{bh}",
                               name=f"va{bh}") for bh in range(NBH)]
            ctxT = [ctxtp.tile([P, 8 * P], IN_DT, tag=f"cx{bh}",
                               name=f"cx{bh}") for bh in range(NBH)]
            wo_t = [[wop.tile([P, 512], IN_DT, tag=f"wo{j}_{oc}",
                              name=f"wo{j}_{oc}") for oc in range(2)]
                    for j in range(8)]
            wo_b = wop.tile([1, D], IN_DT, tag="wo_b")
            ones_f = onesp.tile([1, P], F32, tag="ones_f")
            nc.gpsimd.memset(ones_f[:], 1.0)
            ones1 = onesp.tile([1, P], IN_DT, tag="ones1")
            nc.vector.tensor_copy(ones1[:], ones_f[:])

            # ---------------- phase 1: projections
            with (
                tc.tile_pool(name="xsp", bufs=1) as xsp,
                tc.tile_pool(name="wstr", bufs=1) as wstr,
                tc.tile_pool(name="pp1", bufs=6, space=PSUM) as pp1,
            ):
                xt = {}
                xdram = {"q": xq, "k": xk, "v": xv}
                qn = xsp.tile([P, 8 * TOKC], IN_DT, tag="qn", name="qn")

                # q, k in [col, tok] layout; x tiles on the ACT DMA ring,
                # W row-tiles on the SP ring so loads stream in parallel
                for name, dram in (("q", wq), ("k", wk)):
                    wrow = []
                    for ft in range(FT + 1):
                        kp = P if ft < FT else 1
                        t = xsp.tile([kp, TOKC], IN_DT, tag=f"x{name}{ft}",
                                     name=f"x{name}{ft}")
                        nc.scalar.dma_start(t[:],
                                            xdram[name][ft * P:ft * P + kp, :])
                        xt[name, ft] = t
                        wt = wstr.tile([kp, D], IN_DT, tag=f"w{name}{ft}",
                                       name=f"w{name}{ft}")
                        nc.sync.dma_start(wt[:], dram[ft * P:ft * P + kp, :])
                        wrow.append(wt)
                    for ct in range(8):
                        pt = pp1.tile([P, TOKC], F32, tag="pp1", name="pp1")
                        for ft in range(FT + 1):
                            nc.tensor.matmul(
                                pt[:],
                                wrow[ft][:, ct * P:(ct + 1) * P],
                                xt[name, ft][:],
                                start=(ft == 0), stop=(ft == FT))
                        if name == "q":
                            nc.vector.tensor_copy(
                                qn[:, ct * TOKC:(ct + 1) * TOKC], pt[:])
                        else:
                            nc.vector.tensor_copy(kn[ct][:], pt[:])

                # build q2[bh][{0:64,64:128}, ccq*128:+128] = q chunk ccq
                # of this bh's tokens (both halves identical). One strided
                # DMA per (bh, half, parity) gathers all 8 same-parity
                # chunks: src = qn[par*64:+64, ct*512 + bh*128 (+128)],
                # dst free offsets ccq=2*ct+par -> (2*ct+par)*128.
                qnv = qn.rearrange("p (ct t) -> p ct t", t=TOKC)
                for bh in range(NBH):
                    q2v = q2[bh].rearrange("p (ct pr t) -> p ct pr t",
                                           pr=2, t=P)
                    for r in (0, 64):
                        for par in (0, 1):
                            nc.sync.dma_start(
                                q2v[r:r + 64, :, par, :],
                                qnv[par * 64:par * 64 + 64, :,
                                    bh * P:(bh + 1) * P])

                # v in natural [tok, col] layout, straight into vaug (+ones)
                wvrow = []
                for ft in range(FT + 1):
                    kp = P if ft < FT else 1
                    t = xsp.tile([kp, TOKC], IN_DT, tag=f"xv{ft}",
                                 name=f"xv{ft}")
                    nc.scalar.dma_start(t[:], xv[ft * P:ft * P + kp, :])
                    xt["v", ft] = t
                    wt = wstr.tile([kp, D], IN_DT, tag=f"wv{ft}",
                                   name=f"wv{ft}")
                    nc.sync.dma_start(wt[:], wv[ft * P:ft * P + kp, :])
                    wvrow.append(wt)
                for bh in range(NBH):
                    ts = slice(bh * P, (bh + 1) * P)
                    vview = vaug[bh].rearrange("p (c e) -> p c e", e=DH + 1)
                    for oc in range(2):
                        pt = pp1.tile([P, 512], F32, tag="pp1", name="pp1v")
                        for ft in range(FT + 1):
                            nc.tensor.matmul(pt[:], xt["v", ft][:, ts],
                                             wvrow[ft][:, oc * 512:
                                                       oc * 512 + 512],
                                             start=(ft == 0),
                                             stop=(ft == FT))
                        pview = pt.rearrange("p (c e) -> p c e", e=DH)
                        nc.vector.tensor_copy(
                            vview[:, oc * 8:(oc + 1) * 8, 0:DH], pview[:])
                    nc.gpsimd.memset(vview[:, :, DH:DH + 1], 1.0)

            # ---------------- phase 2: attention
            with (
                tc.tile_pool(name="exps", bufs=3) as exps,
                tc.tile_pool(name="bcp", bufs=4) as bcp,
                tc.tile_pool(name="rzp", bufs=4) as rzp,
                tc.tile_pool(name="ctmp", bufs=4) as ctmp,
                tc.tile_pool(name="osb", bufs=4) as osb,
                tc.tile_pool(name="pps", bufs=2, space=PSUM) as pps,
                tc.tile_pool(name="ppav", bufs=3, space=PSUM) as ppav,
                tc.tile_pool(name="ppo", bufs=1, space=PSUM) as ppo,
            ):
                for j in range(8):
                    for oc in range(2):
                        nc.sync.dma_start(
                            wo_t[j][oc][:],
                            wo[j * P:(j + 1) * P, oc * 512:oc * 512 + 512])
                nc.sync.dma_start(wo_b[:], wo[D:D + 1, :])
                for bh in range(NBH):
                    ts = slice(bh * P, (bh + 1) * P)
                    for g in range(4):
                        pav = ppav.tile([DH + 1, 512], F32, tag="pav",
                                        name="pav")
                        eb = exps.tile([P, NCC * 512], EX_DT, tag="exps",
                                       name="exps")
                        for t in range(8):          # cck pairs 2t, 2t+1
                            pss = pps.tile([P, 1024], F32, tag="pss",
                                           name="pss")
                            for par in (0, 1):
                                cck = 2 * t + par
                                rk = par * 64
                                nc.tensor.matmul(
                                    pss[:, par * 512:par * 512 + 512],
                                    kn[cck // 2][rk:rk + 64, ts],
                                    q2[bh][rk:rk + 64,
                                           g * 512:g * 512 + 512])
                            nc.scalar.activation(
                                eb[:, t * 1024:(t + 1) * 1024], pss[:],
                                Exp, scale=0.125)
                            for par in (0, 1):
                                cck = 2 * t + par
                                nc.tensor.matmul(
                                    pav[:],
                                    vaug[bh][:, cck * (DH + 1):
                                             (cck + 1) * (DH + 1)],
                                    eb[:, cck * 512:(cck + 1) * 512],
                                    start=(cck == 0), stop=(cck == 15))
                        # softmax denominators are row 64 of pav; spread
                        # the 512 Z values across all partitions so the
                        # reciprocal runs 128 lanes wide (a [1,512] slice
                        # would be single-lane and ~3.4us)
                        zrow = rzp.tile([P, 512], F32, tag="zrow",
                                        name="zrow")
                        nc.vector.tensor_copy(zrow[64:65, :], pav[64:65, :])
                        zc = rzp.tile([P, 4], F32, tag="zc", name="zc")
                        nc.sync.dma_start(zc[:], zrow[64:65, :])
                        zr = rzp.tile([P, 4], F32, tag="zr", name="zr")
                        nc.vector.reciprocal_approx_fast(zr[:], zc[:])
                        zrb = rzp.tile([P, 4], EX_DT, tag="zrb", name="zrb")
                        nc.vector.tensor_copy(zrb[:], zr[:])
                        rz0 = rzp.tile([1, 512], EX_DT, tag="rz0",
                                       name="rz0")
                        nc.sync.dma_start(rz0[:], zrb[:])
                        bc = bcp.tile([P, 512], EX_DT, tag="bc", name="bc")
                        nc.gpsimd.partition_broadcast(bc[:], rz0[:])
                        for cck in range(NCC):
                            esl = eb[:, cck * 512:(cck + 1) * 512]
                            nc.vector.tensor_tensor(esl, esl, bc[:], MULT)
                        nc.sync.dma_start(probs[bh, g], eb[:])
                        ct_ = ctmp.tile([DH, 512], IN_DT, tag="ctmp",
                                        name="ctmp")
                        nc.vector.tensor_tensor(ct_[:], pav[0:DH, :],
                                                bc[0:DH, :], MULT)
                        # scatter ct_ chunks into ctxT: ccq=g*4+l ->
                        # tile col-block ccq//2, rows (ccq%2)*64. Group g
                        # covers ct-blocks 2g, 2g+1; parity == l%2.
                        cxv = ctxT[bh].rearrange("p (ct t) -> p ct t", t=P)
                        ctv = ct_.rearrange("p (j pr t) -> p j pr t",
                                            pr=2, t=P)
                        for par in (0, 1):
                            nc.sync.dma_start(
                                cxv[par * 64:par * 64 + 64,
                                    2 * g:2 * g + 2, :],
                                ctv[:, :, par, :])
                    # ---- output projection for this bh block
                    for oc in range(2):
                        po = ppo.tile([P, 512], F32, tag="po", name="po")
                        for j in range(8):
                            nc.tensor.matmul(po[:],
                                             ctxT[bh][:, j * P:(j + 1) * P],
                                             wo_t[j][oc][:],
                                             start=(j == 0), stop=False)
                        nc.tensor.matmul(po[:], ones1[:],
                                         wo_b[:, oc * 512:oc * 512 + 512],
                                         start=False, stop=True)
                        po_sb = osb.tile([P, 512], F32, tag="po_sb",
                                         name="po_sb")
                        if oc == 0:
                            nc.vector.tensor_copy(po_sb[:], po[:])
                        else:
                            nc.scalar.copy(po_sb[:], po[:])
                        nc.sync.dma_start(
                            opart[bh * P:(bh + 1) * P,
                                  oc * 512:oc * 512 + 512], po_sb[:])

    nc.compile()
    return nc


def _get_prog():
    if "nc" not in _prog:
        _prog["nc"] = _build()
    return _prog["nc"]


def kernel(key, value, query, Wk, bk, Wv, bv, Wq, bq, Wo, bo):
    from concourse.bass_utils import run_bass_kernel_spmd

    f32 = np.float32
    in_np = ml_dtypes.bfloat16 if IN_BF16 else f32

    key, value, query = (np.asarray(t, f32) for t in (key, value, query))
    Wk, bk, Wv, bv, Wq, bq, Wo, bo = (
        np.asarray(t, f32) for t in (Wk, bk, Wv, bv, Wq, bq, Wo, bo))

    xf = {"xq": query.reshape(B * L, D), "xk": key.reshape(B * L, D),
          "xv": value.reshape(B * L, D)}
    w_aug = {"wq": np.concatenate([Wq, bq[None, :]], 0).astype(in_np),
             "wk": np.concatenate([Wk, bk[None, :]], 0).astype(in_np),
             "wv": np.concatenate([Wv, bv[None, :]], 0).astype(in_np),
             "wo": np.concatenate([Wo, bo[None, :]], 0).astype(in_np)}

    ones = np.ones((1, TOKC), f32)
    in_maps = []
    for c in range(N_CORES):
        rows = np.concatenate([
            np.arange((bh // HPC) * L + (HPC * c + bh % HPC) * P,
                      (bh // HPC) * L + (HPC * c + bh % HPC) * P + P)
            for bh in range(NBH)])
        m = dict(w_aug)
        for n in ("xq", "xk", "xv"):
            m[n] = np.ascontiguousarray(
                np.concatenate([xf[n][rows].T, ones], 0)).astype(in_np)
        in_maps.append(m)

    nc = _get_prog()
    res = run_bass_kernel_spmd(nc, in_maps, list(range(N_CORES)),
                               trace=False)

    attention = np.empty((B * H, L, L), f32)
    context = np.empty((B, L, D), f32)
    for c in range(N_CORES):
        r = res.results[c]
        # probs [bh, g, tk, (cck, ccq_l, tq)] -> [bh, mq, mk]
        p = np.asarray(r["probs"]).astype(f32)
        p = p.reshape(NBH, 4, P, NCC, 4, P)
        p = p.transpose(0, 5, 1, 4, 2, 3).reshape(NBH, L, L)
        op = np.asarray(r["opart"], f32)
        for bh in range(NBH):
            b, hl = bh // HPC, bh % HPC
            h = HPC * c + hl
            attention[b * H + h] = p[bh]
            context[b, h * P:(h + 1) * P, :] = op[bh * P:(bh + 1) * P, :]
    return context, attention


# revision 24
# speedup vs baseline: 1.5574x; 1.0548x over previous
"""nn_MultiHeadAttention on 8 TRN2 NeuronCores.

IMPORTANT semantics: the reference does a RAW reshape (torch .view style)
  k.reshape(B*H, Lk, d)   with k = [B, L, D] and H*d = D, L = 2048, d = 64.
Since L*D = H * (L*d), "head" i = (b, h) is NOT a feature slice: it is the
contiguous slab = tokens [h*128, (h+1)*128) of batch b, with those 128
tokens' full 1024-dim projected features re-chunked into 2048 virtual
positions m = tok_local*16 + cc, each of 64 dims (cc = feature chunk).

Sharding: token/head-parallel. Core c owns heads {2c, 2c+1} x both batches
= 4 blocks of 128 tokens (bh-major order, bh = b*2 + hl). Everything is
local to the core: projections for its 512 tokens (full D columns),
attention for its 4 heads, and its 512 rows of the output projection.
Host just concatenates token slices (and unscrambles the probs layout).

Device layout per core:
  xT      [1025, 512]  x^T slice + ones row (projection bias via matmul)
  qn/kn   8 tiles [128(col), 512(tok)] f32r  - transposed projections
  qs      8 tiles - qn with chunks shifted one position (SBUF->SBUF DMA)
           so any (cck, ccq) parity pair can share array row groups
  vn -> vaug per bh: [128(tok), 16*65] bf16: per chunk 64 v-cols + ones col
  scores^T blocks [tk=128, tq=128] via K=64 matmuls, 4 per PSUM bank
  softmax denominator = ones-column row of the PV matmul (row 64 of PSUM)
  probs written as [bh, g, cck, tk, (ccq_l, tq)] - host permutes
  o-proj: ctx^T tiles [128(col), 128(tok)] (placed via SBUF->SBUF DMA)
          @ Wo tiles, bias row via ones matmul. opart [512, 1024] local.
"""

import sys

_REPO = "/opt/trn_rl_repo"
if _REPO not in sys.path:
    sys.path.insert(0, _REPO)

import numpy as np
import ml_dtypes

# ---------------------------------------------------------------- config
EXPS_BF16 = True      # exp(scores)/probs-out/v_aug in bf16 (else f32r)
IN_BF16 = True       # activations/weights bf16 (else f32 + f32r matmuls)

B, L, D = 2, 2048, 1024
H, DH = 16, 64
HPC = 2                       # heads per core
N_CORES = 8
TOKC = 512                    # tokens per core (4 bh-blocks of 128)
NBH = B * HPC                 # 4 local (batch, head) units
P = 128
FT = 8                        # full 128-row feature tiles (+1 bias row)
NCC = 16                      # feature chunks of 64 (virtual positions)

_prog = {}


def _build():
    import concourse.bass as bass
    import concourse.bacc as bacc
    import concourse.mybir as mybir
    import concourse.tile as tile

    F32 = mybir.dt.float32
    F32R = mybir.dt.float32r
    BF16 = mybir.dt.bfloat16
    Exp = mybir.ActivationFunctionType.Exp
    MULT = mybir.AluOpType.mult
    PSUM = bass.MemorySpace.PSUM

    IN_DT = BF16 if IN_BF16 else F32R
    EX_DT = BF16 if EXPS_BF16 else F32R

    nc = bacc.Bacc("TRN2", target_bir_lowering=False, debug=False,
                   num_devices=N_CORES)

    xq = nc.declare_dram_parameter("xq", [D + 1, TOKC], IN_DT, isOutput=False)
    xk = nc.declare_dram_parameter("xk", [D + 1, TOKC], IN_DT, isOutput=False)
    xv = nc.declare_dram_parameter("xv", [D + 1, TOKC], IN_DT, isOutput=False)
    wq = nc.declare_dram_parameter("wq", [D + 1, D], IN_DT, isOutput=False)
    wk = nc.declare_dram_parameter("wk", [D + 1, D], IN_DT, isOutput=False)
    wv = nc.declare_dram_parameter("wv", [D + 1, D], IN_DT, isOutput=False)
    wo = nc.declare_dram_parameter("wo", [D + 1, D], IN_DT, isOutput=False)
    probs = nc.declare_dram_parameter("probs", [NBH, 4, P, NCC * 512], EX_DT,
                                      isOutput=True)
    opart = nc.declare_dram_parameter("opart", [TOKC, D], F32, isOutput=True)

    with tile.TileContext(nc) as tc:
        with (
            tc.tile_pool(name="qkst", bufs=1) as qkst,
            tc.tile_pool(name="vaugp", bufs=1) as vaugp,
            tc.tile_pool(name="ctxtp", bufs=1) as ctxtp,
            tc.tile_pool(name="wop", bufs=1) as wop,
            tc.tile_pool(name="onesp", bufs=1) as onesp,
        ):
            kn = [qkst.tile([P, TOKC], IN_DT, tag=f"kn{j}", name=f"kn{j}")
                  for j in range(8)]
            # per-bh q^T in head-sequence order [dd, mq=(ccq,tq)],
            # duplicated to both partition halves so it can pair with
            # either row-group parity of the k chunks
            q2 = [qkst.tile([P, L], IN_DT, tag=f"q2{bh}", name=f"q2{bh}")
                  for bh in range(NBH)]
            vaug = [vaugp.tile([P, NCC * (DH + 1)], EX_DT, tag=f"va{bh}",
                               name=f"va{bh}") for bh in range(NBH)]
            ctxT = [ctxtp.tile([P, 8 * P], IN_DT, tag=f"cx{bh}",
                               name=f"cx{bh}") for bh in range(NBH)]
            wo_t = [[wop.tile([P, 512], IN_DT, tag=f"wo{j}_{oc}",
                              name=f"wo{j}_{oc}") for oc in range(2)]
                    for j in range(8)]
            wo_b = wop.tile([1, D], IN_DT, tag="wo_b")
            ones_f = onesp.tile([1, P], F32, tag="ones_f")
            nc.gpsimd.memset(ones_f[:], 1.0)
            ones1 = onesp.tile([1, P], IN_DT, tag="ones1")
            nc.vector.tensor_copy(ones1[:], ones_f[:])

            # ---------------- phase 1: projections
            with (
                tc.tile_pool(name="xsp", bufs=1) as xsp,
                tc.tile_pool(name="wstr", bufs=1) as wstr,
                tc.tile_pool(name="pp1", bufs=6, space=PSUM) as pp1,
            ):
                xt = {}
                xdram = {"q": xq, "k": xk, "v": xv}
                qn = xsp.tile([P, 8 * TOKC], IN_DT, tag="qn", name="qn")

                # q, k in [col, tok] layout; x tiles on the ACT DMA ring,
                # W row-tiles on the SP ring so loads stream in parallel
                for name, dram in (("q", wq), ("k", wk)):
                    wrow = []
                    for ft in range(FT + 1):
                        kp = P if ft < FT else 1
                        t = xsp.tile([kp, TOKC], IN_DT, tag=f"x{name}{ft}",
                                     name=f"x{name}{ft}")
                        nc.scalar.dma_start(t[:],
                                            xdram[name][ft * P:ft * P + kp, :])
                        xt[name, ft] = t
                        wt = wstr.tile([kp, D], IN_DT, tag=f"w{name}{ft}",
                                       name=f"w{name}{ft}")
                        nc.sync.dma_start(wt[:], dram[ft * P:ft * P + kp, :])
                        wrow.append(wt)
                    for ct in range(8):
                        pt = pp1.tile([P, TOKC], F32, tag="pp1", name="pp1")
                        for ft in range(FT + 1):
                            nc.tensor.matmul(
                                pt[:],
                                wrow[ft][:, ct * P:(ct + 1) * P],
                                xt[name, ft][:],
                                start=(ft == 0), stop=(ft == FT))
                        if name == "q":
                            nc.vector.tensor_copy(
                                qn[:, ct * TOKC:(ct + 1) * TOKC], pt[:])
                        else:
                            nc.vector.tensor_copy(kn[ct][:], pt[:])

                # build q2[bh][{0:64,64:128}, ccq*128:+128] = q chunk ccq
                # of this bh's tokens (both halves identical). One strided
                # DMA per (bh, half, parity) gathers all 8 same-parity
                # chunks: src = qn[par*64:+64, ct*512 + bh*128 (+128)],
                # dst free offsets ccq=2*ct+par -> (2*ct+par)*128.
                qnv = qn.rearrange("p (ct t) -> p ct t", t=TOKC)
                for bh in range(NBH):
                    q2v = q2[bh].rearrange("p (ct pr t) -> p ct pr t",
                                           pr=2, t=P)
                    for r in (0, 64):
                        for par in (0, 1):
                            nc.sync.dma_start(
                                q2v[r:r + 64, :, par, :],
                                qnv[par * 64:par * 64 + 64, :,
                                    bh * P:(bh + 1) * P])

                # v in natural [tok, col] layout, straight into vaug (+ones)
                wvrow = []
                for ft in range(FT + 1):
                    kp = P if ft < FT else 1
                    t = xsp.tile([kp, TOKC], IN_DT, tag=f"xv{ft}",
                                 name=f"xv{ft}")
                    nc.scalar.dma_start(t[:], xv[ft * P:ft * P + kp, :])
                    xt["v", ft] = t
                    wt = wstr.tile([kp, D], IN_DT, tag=f"wv{ft}",
                                   name=f"wv{ft}")
                    nc.sync.dma_start(wt[:], wv[ft * P:ft * P + kp, :])
                    wvrow.append(wt)
                for bh in range(NBH):
                    ts = slice(bh * P, (bh + 1) * P)
                    vview = vaug[bh].rearrange("p (c e) -> p c e", e=DH + 1)
                    for oc in range(2):
                        pt = pp1.tile([P, 512], F32, tag="pp1", name="pp1v")
                        for ft in range(FT + 1):
                            nc.tensor.matmul(pt[:], xt["v", ft][:, ts],
                                             wvrow[ft][:, oc * 512:
                                                       oc * 512 + 512],
                                             start=(ft == 0),
                                             stop=(ft == FT))
                        pview = pt.rearrange("p (c e) -> p c e", e=DH)
                        nc.vector.tensor_copy(
                            vview[:, oc * 8:(oc + 1) * 8, 0:DH], pview[:])
                    nc.gpsimd.memset(vview[:, :, DH:DH + 1], 1.0)

            # ---------------- phase 2: attention
            with (
                tc.tile_pool(name="exps", bufs=4) as exps,
                tc.tile_pool(name="bcp", bufs=4) as bcp,
                tc.tile_pool(name="rzp", bufs=4) as rzp,
                tc.tile_pool(name="ctmp", bufs=4) as ctmp,
                tc.tile_pool(name="osb", bufs=4) as osb,
                tc.tile_pool(name="pps", bufs=2, space=PSUM) as pps,
                tc.tile_pool(name="ppav", bufs=3, space=PSUM) as ppav,
                tc.tile_pool(name="ppo", bufs=1, space=PSUM) as ppo,
            ):
                for j in range(8):
                    for oc in range(2):
                        nc.sync.dma_start(
                            wo_t[j][oc][:],
                            wo[j * P:(j + 1) * P, oc * 512:oc * 512 + 512])
                nc.sync.dma_start(wo_b[:], wo[D:D + 1, :])
                for bh in range(NBH):
                    ts = slice(bh * P, (bh + 1) * P)
                    for g in range(4):
                        pav = ppav.tile([DH + 1, 512], F32, tag="pav",
                                        name="pav")
                        eb = exps.tile([P, NCC * 512], EX_DT, tag="exps",
                                       name="exps")
                        for t in range(8):          # cck pairs 2t, 2t+1
                            pss = pps.tile([P, 1024], F32, tag="pss",
                                           name="pss")
                            for par in (0, 1):
                                cck = 2 * t + par
                                rk = par * 64
                                nc.tensor.matmul(
                                    pss[:, par * 512:par * 512 + 512],
                                    kn[cck // 2][rk:rk + 64, ts],
                                    q2[bh][rk:rk + 64,
                                           g * 512:g * 512 + 512])
                            nc.scalar.activation(
                                eb[:, t * 1024:(t + 1) * 1024], pss[:],
                                Exp, scale=0.125)
                            for par in (0, 1):
                                cck = 2 * t + par
                                nc.tensor.matmul(
                                    pav[:],
                                    vaug[bh][:, cck * (DH + 1):
                                             (cck + 1) * (DH + 1)],
                                    eb[:, cck * 512:(cck + 1) * 512],
                                    start=(cck == 0), stop=(cck == 15))
                        # softmax denominators are row 64 of pav; spread
                        # the 512 Z values across all partitions so the
                        # reciprocal runs 128 lanes wide (a [1,512] slice
                        # would be single-lane and ~3.4us)
                        zrow = rzp.tile([P, 512], F32, tag="zrow",
                                        name="zrow")
                        nc.vector.tensor_copy(zrow[64:65, :], pav[64:65, :])
                        zc = rzp.tile([P, 4], F32, tag="zc", name="zc")
                        nc.sync.dma_start(zc[:], zrow[64:65, :])
                        zr = rzp.tile([P, 4], F32, tag="zr", name="zr")
                        nc.vector.reciprocal_approx_fast(zr[:], zc[:])
                        zrb = rzp.tile([P, 4], EX_DT, tag="zrb", name="zrb")
                        nc.vector.tensor_copy(zrb[:], zr[:])
                        rz0 = rzp.tile([1, 512], EX_DT, tag="rz0",
                                       name="rz0")
                        nc.sync.dma_start(rz0[:], zrb[:])
                        bc = bcp.tile([P, 512], EX_DT, tag="bc", name="bc")
                        nc.gpsimd.partition_broadcast(bc[:], rz0[:])
                        for cck in range(NCC):
                            esl = eb[:, cck * 512:(cck + 1) * 512]
                            nc.vector.tensor_tensor(esl, esl, bc[:], MULT)
                        nc.sync.dma_start(probs[bh, g], eb[:])
                        ct_ = ctmp.tile([DH, 512], IN_DT, tag="ctmp",
                                        name="ctmp")
                        nc.vector.tensor_tensor(ct_[:], pav[0:DH, :],
                                                bc[0:DH, :], MULT)
                        # scatter ct_ chunks into ctxT: ccq=g*4+l ->
                        # tile col-block ccq//2, rows (ccq%2)*64. Group g
                        # covers ct-blocks 2g, 2g+1; parity == l%2.
                        cxv = ctxT[bh].rearrange("p (ct t) -> p ct t", t=P)
                        ctv = ct_.rearrange("p (j pr t) -> p j pr t",
                                            pr=2, t=P)
                        for par in (0, 1):
                            nc.sync.dma_start(
                                cxv[par * 64:par * 64 + 64,
                                    2 * g:2 * g + 2, :],
                                ctv[:, :, par, :])
                    # ---- output projection for this bh block
                    for oc in range(2):
                        po = ppo.tile([P, 512], F32, tag="po", name="po")
                        for j in range(8):
                            nc.tensor.matmul(po[:],
                                             ctxT[bh][:, j * P:(j + 1) * P],
                                             wo_t[j][oc][:],
                                             start=(j == 0), stop=False)
                        nc.tensor.matmul(po[:], ones1[:],
                                         wo_b[:, oc * 512:oc * 512 + 512],
                                         start=False, stop=True)
                        po_sb = osb.tile([P, 512], F32, tag="po_sb",
                                         name="po_sb")
                        if oc == 0:
                            nc.vector.tensor_copy(po_sb[:], po[:])
                        else:
                            nc.scalar.copy(po_sb[:], po[:])
                        nc.sync.dma_start(
                            opart[bh * P:(bh + 1) * P,
                                  oc * 512:oc * 512 + 512], po_sb[:])

    nc.compile()
    return nc


def _get_prog():
    if "nc" not in _prog:
        _prog["nc"] = _build()
    return _prog["nc"]


def kernel(key, value, query, Wk, bk, Wv, bv, Wq, bq, Wo, bo):
    from concourse.bass_utils import run_bass_kernel_spmd

    f32 = np.float32
    in_np = ml_dtypes.bfloat16 if IN_BF16 else f32

    key, value, query = (np.asarray(t, f32) for t in (key, value, query))
    Wk, bk, Wv, bv, Wq, bq, Wo, bo = (
        np.asarray(t, f32) for t in (Wk, bk, Wv, bv, Wq, bq, Wo, bo))

    xf = {"xq": query.reshape(B * L, D), "xk": key.reshape(B * L, D),
          "xv": value.reshape(B * L, D)}
    w_aug = {"wq": np.concatenate([Wq, bq[None, :]], 0).astype(in_np),
             "wk": np.concatenate([Wk, bk[None, :]], 0).astype(in_np),
             "wv": np.concatenate([Wv, bv[None, :]], 0).astype(in_np),
             "wo": np.concatenate([Wo, bo[None, :]], 0).astype(in_np)}

    ones = np.ones((1, TOKC), f32)
    in_maps = []
    for c in range(N_CORES):
        rows = np.concatenate([
            np.arange((bh // HPC) * L + (HPC * c + bh % HPC) * P,
                      (bh // HPC) * L + (HPC * c + bh % HPC) * P + P)
            for bh in range(NBH)])
        m = dict(w_aug)
        for n in ("xq", "xk", "xv"):
            m[n] = np.ascontiguousarray(
                np.concatenate([xf[n][rows].T, ones], 0)).astype(in_np)
        in_maps.append(m)

    nc = _get_prog()
    res = run_bass_kernel_spmd(nc, in_maps, list(range(N_CORES)),
                               trace=False)

    attention = np.empty((B * H, L, L), f32)
    context = np.empty((B, L, D), f32)
    for c in range(N_CORES):
        r = res.results[c]
        # probs [bh, g, tk, (cck, ccq_l, tq)] -> [bh, mq, mk]
        p = np.asarray(r["probs"]).astype(f32)
        p = p.reshape(NBH, 4, P, NCC, 4, P)
        p = p.transpose(0, 5, 1, 4, 2, 3).reshape(NBH, L, L)
        op = np.asarray(r["opart"], f32)
        for bh in range(NBH):
            b, hl = bh // HPC, bh % HPC
            h = HPC * c + hl
            attention[b * H + h] = p[bh]
            context[b, h * P:(h + 1) * P, :] = op[bh * P:(bh + 1) * P, :]
    return context, attention


# revision 25
# speedup vs baseline: 1.6494x; 1.0591x over previous
"""nn_MultiHeadAttention on 8 TRN2 NeuronCores.

IMPORTANT semantics: the reference does a RAW reshape (torch .view style)
  k.reshape(B*H, Lk, d)   with k = [B, L, D] and H*d = D, L = 2048, d = 64.
Since L*D = H * (L*d), "head" i = (b, h) is NOT a feature slice: it is the
contiguous slab = tokens [h*128, (h+1)*128) of batch b, with those 128
tokens' full 1024-dim projected features re-chunked into 2048 virtual
positions m = tok_local*16 + cc, each of 64 dims (cc = feature chunk).

Sharding: token/head-parallel. Core c owns heads {2c, 2c+1} x both batches
= 4 blocks of 128 tokens (bh-major order, bh = b*2 + hl). Everything is
local to the core: projections for its 512 tokens (full D columns),
attention for its 4 heads, and its 512 rows of the output projection.
Host just concatenates token slices (and unscrambles the probs layout).

Device layout per core:
  xT      [1025, 512]  x^T slice + ones row (projection bias via matmul)
  qn/kn   8 tiles [128(col), 512(tok)] f32r  - transposed projections
  qs      8 tiles - qn with chunks shifted one position (SBUF->SBUF DMA)
           so any (cck, ccq) parity pair can share array row groups
  vn -> vaug per bh: [128(tok), 16*65] bf16: per chunk 64 v-cols + ones col
  scores^T blocks [tk=128, tq=128] via K=64 matmuls, 4 per PSUM bank
  softmax denominator = ones-column row of the PV matmul (row 64 of PSUM)
  probs written as [bh, g, cck, tk, (ccq_l, tq)] - host permutes
  o-proj: ctx^T tiles [128(col), 128(tok)] (placed via SBUF->SBUF DMA)
          @ Wo tiles, bias row via ones matmul. opart [512, 1024] local.
"""

import sys

_REPO = "/opt/trn_rl_repo"
if _REPO not in sys.path:
    sys.path.insert(0, _REPO)

import numpy as np
import ml_dtypes

# ---------------------------------------------------------------- config
EXPS_BF16 = True      # exp(scores)/probs-out/v_aug in bf16 (else f32r)
IN_BF16 = True       # activations/weights bf16 (else f32 + f32r matmuls)

B, L, D = 2, 2048, 1024
H, DH = 16, 64
HPC = 2                       # heads per core
N_CORES = 8
TOKC = 512                    # tokens per core (4 bh-blocks of 128)
NBH = B * HPC                 # 4 local (batch, head) units
P = 128
FT = 8                        # full 128-row feature tiles (+1 bias row)
NCC = 16                      # feature chunks of 64 (virtual positions)

_prog = {}


def _build():
    import concourse.bass as bass
    import concourse.bacc as bacc
    import concourse.mybir as mybir
    import concourse.tile as tile

    F32 = mybir.dt.float32
    F32R = mybir.dt.float32r
    BF16 = mybir.dt.bfloat16
    Exp = mybir.ActivationFunctionType.Exp
    MULT = mybir.AluOpType.mult
    PSUM = bass.MemorySpace.PSUM

    IN_DT = BF16 if IN_BF16 else F32R
    EX_DT = BF16 if EXPS_BF16 else F32R

    nc = bacc.Bacc("TRN2", target_bir_lowering=False, debug=False,
                   num_devices=N_CORES)

    xq = nc.declare_dram_parameter("xq", [D + 1, TOKC], IN_DT, isOutput=False)
    xk = nc.declare_dram_parameter("xk", [D + 1, TOKC], IN_DT, isOutput=False)
    xv = nc.declare_dram_parameter("xv", [D + 1, TOKC], IN_DT, isOutput=False)
    wq = nc.declare_dram_parameter("wq", [D + 1, D], IN_DT, isOutput=False)
    wk = nc.declare_dram_parameter("wk", [D + 1, D], IN_DT, isOutput=False)
    wv = nc.declare_dram_parameter("wv", [D + 1, D], IN_DT, isOutput=False)
    wo = nc.declare_dram_parameter("wo", [D + 1, D], IN_DT, isOutput=False)
    probs = nc.declare_dram_parameter("probs", [NBH, 4, P, NCC * 512], EX_DT,
                                      isOutput=True)
    opart = nc.declare_dram_parameter("opart", [TOKC, D], F32, isOutput=True)

    with tile.TileContext(nc) as tc:
        with (
            tc.tile_pool(name="qkst", bufs=1) as qkst,
            tc.tile_pool(name="vaugp", bufs=1) as vaugp,
            tc.tile_pool(name="ctxtp", bufs=1) as ctxtp,
            tc.tile_pool(name="wop", bufs=1) as wop,
            tc.tile_pool(name="onesp", bufs=1) as onesp,
        ):
            kn = [qkst.tile([P, TOKC], IN_DT, tag=f"kn{j}", name=f"kn{j}")
                  for j in range(8)]
            # per-bh q^T in head-sequence order [dd, mq=(ccq,tq)],
            # duplicated to both partition halves so it can pair with
            # either row-group parity of the k chunks
            q2 = [qkst.tile([P, L], IN_DT, tag=f"q2{bh}", name=f"q2{bh}")
                  for bh in range(NBH)]
            vaug = [vaugp.tile([P, NCC * (DH + 1)], EX_DT, tag=f"va{bh}",
                               name=f"va{bh}") for bh in range(NBH)]
            ctxT = [ctxtp.tile([P, 8 * P], IN_DT, tag=f"cx{bh}",
                               name=f"cx{bh}") for bh in range(NBH)]
            wo_t = [[wop.tile([P, 512], IN_DT, tag=f"wo{j}_{oc}",
                              name=f"wo{j}_{oc}") for oc in range(2)]
                    for j in range(8)]
            wo_b = wop.tile([1, D], IN_DT, tag="wo_b")
            ones_f = onesp.tile([1, P], F32, tag="ones_f")
            nc.gpsimd.memset(ones_f[:], 1.0)
            ones1 = onesp.tile([1, P], IN_DT, tag="ones1")
            nc.vector.tensor_copy(ones1[:], ones_f[:])

            # ---------------- phase 1: projections
            with (
                tc.tile_pool(name="xsp", bufs=1) as xsp,
                tc.tile_pool(name="wstr", bufs=1) as wstr,
                tc.tile_pool(name="pp1", bufs=6, space=PSUM) as pp1,
            ):
                xt = {}
                xdram = {"q": xq, "k": xk, "v": xv}
                qn = xsp.tile([P, 8 * TOKC], IN_DT, tag="qn", name="qn")

                # q, k in [col, tok] layout; x tiles on the ACT DMA ring,
                # W row-tiles on the SP ring so loads stream in parallel
                for name, dram in (("q", wq), ("k", wk)):
                    wrow = []
                    for ft in range(FT + 1):
                        kp = P if ft < FT else 1
                        t = xsp.tile([kp, TOKC], IN_DT, tag=f"x{name}{ft}",
                                     name=f"x{name}{ft}")
                        nc.scalar.dma_start(t[:],
                                            xdram[name][ft * P:ft * P + kp, :])
                        xt[name, ft] = t
                        wt = wstr.tile([kp, D], IN_DT, tag=f"w{name}{ft}",
                                       name=f"w{name}{ft}")
                        nc.sync.dma_start(wt[:], dram[ft * P:ft * P + kp, :])
                        wrow.append(wt)
                    for ct in range(8):
                        pt = pp1.tile([P, TOKC], F32, tag="pp1", name="pp1")
                        for ft in range(FT + 1):
                            nc.tensor.matmul(
                                pt[:],
                                wrow[ft][:, ct * P:(ct + 1) * P],
                                xt[name, ft][:],
                                start=(ft == 0), stop=(ft == FT))
                        if name == "q":
                            nc.vector.tensor_copy(
                                qn[:, ct * TOKC:(ct + 1) * TOKC], pt[:])
                        else:
                            nc.vector.tensor_copy(kn[ct][:], pt[:])

                # build q2[bh][{0:64,64:128}, ccq*128:+128] = q chunk ccq
                # of this bh's tokens (both halves identical). One strided
                # DMA per (bh, half, parity) gathers all 8 same-parity
                # chunks: src = qn[par*64:+64, ct*512 + bh*128 (+128)],
                # dst free offsets ccq=2*ct+par -> (2*ct+par)*128.
                qnv = qn.rearrange("p (ct t) -> p ct t", t=TOKC)
                for bh in range(NBH):
                    q2v = q2[bh].rearrange("p (ct pr t) -> p ct pr t",
                                           pr=2, t=P)
                    for r in (0, 64):
                        for par in (0, 1):
                            nc.sync.dma_start(
                                q2v[r:r + 64, :, par, :],
                                qnv[par * 64:par * 64 + 64, :,
                                    bh * P:(bh + 1) * P])

                # v in natural [tok, col] layout, straight into vaug (+ones)
                wvrow = []
                for ft in range(FT + 1):
                    kp = P if ft < FT else 1
                    t = xsp.tile([kp, TOKC], IN_DT, tag=f"xv{ft}",
                                 name=f"xv{ft}")
                    nc.scalar.dma_start(t[:], xv[ft * P:ft * P + kp, :])
                    xt["v", ft] = t
                    wt = wstr.tile([kp, D], IN_DT, tag=f"wv{ft}",
                                   name=f"wv{ft}")
                    nc.sync.dma_start(wt[:], wv[ft * P:ft * P + kp, :])
                    wvrow.append(wt)
                for bh in range(NBH):
                    ts = slice(bh * P, (bh + 1) * P)
                    vview = vaug[bh].rearrange("p (c e) -> p c e", e=DH + 1)
                    for oc in range(2):
                        pt = pp1.tile([P, 512], F32, tag="pp1", name="pp1v")
                        for ft in range(FT + 1):
                            nc.tensor.matmul(pt[:], xt["v", ft][:, ts],
                                             wvrow[ft][:, oc * 512:
                                                       oc * 512 + 512],
                                             start=(ft == 0),
                                             stop=(ft == FT))
                        pview = pt.rearrange("p (c e) -> p c e", e=DH)
                        nc.vector.tensor_copy(
                            vview[:, oc * 8:(oc + 1) * 8, 0:DH], pview[:])
                    nc.gpsimd.memset(vview[:, :, DH:DH + 1], 1.0)

            # ---------------- phase 2: attention
            with (
                tc.tile_pool(name="exps", bufs=6) as exps,
                tc.tile_pool(name="bcp", bufs=4) as bcp,
                tc.tile_pool(name="rzp", bufs=4) as rzp,
                tc.tile_pool(name="ctmp", bufs=4) as ctmp,
                tc.tile_pool(name="osb", bufs=4) as osb,
                tc.tile_pool(name="pps", bufs=2, space=PSUM) as pps,
                tc.tile_pool(name="ppav", bufs=3, space=PSUM) as ppav,
                tc.tile_pool(name="ppo", bufs=1, space=PSUM) as ppo,
            ):
                for j in range(8):
                    for oc in range(2):
                        nc.sync.dma_start(
                            wo_t[j][oc][:],
                            wo[j * P:(j + 1) * P, oc * 512:oc * 512 + 512])
                nc.sync.dma_start(wo_b[:], wo[D:D + 1, :])
                for bh in range(NBH):
                    ts = slice(bh * P, (bh + 1) * P)
                    for g in range(4):
                        pav = ppav.tile([DH + 1, 512], F32, tag="pav",
                                        name="pav")
                        eb = exps.tile([P, NCC * 512], EX_DT, tag="exps",
                                       name="exps")
                        for t in range(8):          # cck pairs 2t, 2t+1
                            pss = pps.tile([P, 1024], F32, tag="pss",
                                           name="pss")
                            for par in (0, 1):
                                cck = 2 * t + par
                                rk = par * 64
                                nc.tensor.matmul(
                                    pss[:, par * 512:par * 512 + 512],
                                    kn[cck // 2][rk:rk + 64, ts],
                                    q2[bh][rk:rk + 64,
                                           g * 512:g * 512 + 512])
                            nc.scalar.activation(
                                eb[:, t * 1024:(t + 1) * 1024], pss[:],
                                Exp, scale=0.125)
                            for par in (0, 1):
                                cck = 2 * t + par
                                nc.tensor.matmul(
                                    pav[:],
                                    vaug[bh][:, cck * (DH + 1):
                                             (cck + 1) * (DH + 1)],
                                    eb[:, cck * 512:(cck + 1) * 512],
                                    start=(cck == 0), stop=(cck == 15))
                        # softmax denominators are row 64 of pav; spread
                        # the 512 Z values across all partitions so the
                        # reciprocal runs 128 lanes wide (a [1,512] slice
                        # would be single-lane and ~3.4us)
                        zrow = rzp.tile([P, 512], F32, tag="zrow",
                                        name="zrow")
                        nc.vector.tensor_copy(zrow[64:65, :], pav[64:65, :])
                        zc = rzp.tile([P, 4], F32, tag="zc", name="zc")
                        nc.sync.dma_start(zc[:], zrow[64:65, :])
                        zr = rzp.tile([P, 4], F32, tag="zr", name="zr")
                        nc.vector.reciprocal_approx_fast(zr[:], zc[:])
                        zrb = rzp.tile([P, 4], EX_DT, tag="zrb", name="zrb")
                        nc.vector.tensor_copy(zrb[:], zr[:])
                        rz0 = rzp.tile([1, 512], EX_DT, tag="rz0",
                                       name="rz0")
                        nc.sync.dma_start(rz0[:], zrb[:])
                        bc = bcp.tile([P, 512], EX_DT, tag="bc", name="bc")
                        nc.gpsimd.partition_broadcast(bc[:], rz0[:])
                        for cck in range(NCC):
                            esl = eb[:, cck * 512:(cck + 1) * 512]
                            nc.vector.tensor_tensor(esl, esl, bc[:], MULT)
                        nc.sync.dma_start(probs[bh, g], eb[:])
                        ct_ = ctmp.tile([DH, 512], IN_DT, tag="ctmp",
                                        name="ctmp")
                        nc.vector.tensor_tensor(ct_[:], pav[0:DH, :],
                                                bc[0:DH, :], MULT)
                        # scatter ct_ chunks into ctxT: ccq=g*4+l ->
                        # tile col-block ccq//2, rows (ccq%2)*64. Group g
                        # covers ct-blocks 2g, 2g+1; parity == l%2.
                        cxv = ctxT[bh].rearrange("p (ct t) -> p ct t", t=P)
                        ctv = ct_.rearrange("p (j pr t) -> p j pr t",
                                            pr=2, t=P)
                        for par in (0, 1):
                            nc.sync.dma_start(
                                cxv[par * 64:par * 64 + 64,
                                    2 * g:2 * g + 2, :],
                                ctv[:, :, par, :])
                    # ---- output projection for this bh block
                    for oc in range(2):
                        po = ppo.tile([P, 512], F32, tag="po", name="po")
                        for j in range(8):
                            nc.tensor.matmul(po[:],
                                             ctxT[bh][:, j * P:(j + 1) * P],
                                             wo_t[j][oc][:],
                                             start=(j == 0), stop=False)
                        nc.tensor.matmul(po[:], ones1[:],
                                         wo_b[:, oc * 512:oc * 512 + 512],
                                         start=False, stop=True)
                        po_sb = osb.tile([P, 512], F32, tag="po_sb",
                                         name="po_sb")
                        if oc == 0:
                            nc.vector.tensor_copy(po_sb[:], po[:])
                        else:
                            nc.scalar.copy(po_sb[:], po[:])
                        nc.sync.dma_start(
                            opart[bh * P:(bh + 1) * P,
                                  oc * 512:oc * 512 + 512], po_sb[:])

    nc.compile()
    return nc


def _get_prog():
    if "nc" not in _prog:
        _prog["nc"] = _build()
    return _prog["nc"]


def kernel(key, value, query, Wk, bk, Wv, bv, Wq, bq, Wo, bo):
    from concourse.bass_utils import run_bass_kernel_spmd

    f32 = np.float32
    in_np = ml_dtypes.bfloat16 if IN_BF16 else f32

    key, value, query = (np.asarray(t, f32) for t in (key, value, query))
    Wk, bk, Wv, bv, Wq, bq, Wo, bo = (
        np.asarray(t, f32) for t in (Wk, bk, Wv, bv, Wq, bq, Wo, bo))

    xf = {"xq": query.reshape(B * L, D), "xk": key.reshape(B * L, D),
          "xv": value.reshape(B * L, D)}
    w_aug = {"wq": np.concatenate([Wq, bq[None, :]], 0).astype(in_np),
             "wk": np.concatenate([Wk, bk[None, :]], 0).astype(in_np),
             "wv": np.concatenate([Wv, bv[None, :]], 0).astype(in_np),
             "wo": np.concatenate([Wo, bo[None, :]], 0).astype(in_np)}

    ones = np.ones((1, TOKC), f32)
    in_maps = []
    for c in range(N_CORES):
        rows = np.concatenate([
            np.arange((bh // HPC) * L + (HPC * c + bh % HPC) * P,
                      (bh // HPC) * L + (HPC * c + bh % HPC) * P + P)
            for bh in range(NBH)])
        m = dict(w_aug)
        for n in ("xq", "xk", "xv"):
            m[n] = np.ascontiguousarray(
                np.concatenate([xf[n][rows].T, ones], 0)).astype(in_np)
        in_maps.append(m)

    nc = _get_prog()
    res = run_bass_kernel_spmd(nc, in_maps, list(range(N_CORES)),
                               trace=False)

    attention = np.empty((B * H, L, L), f32)
    context = np.empty((B, L, D), f32)
    for c in range(N_CORES):
        r = res.results[c]
        # probs [bh, g, tk, (cck, ccq_l, tq)] -> [bh, mq, mk]
        p = np.asarray(r["probs"]).astype(f32)
        p = p.reshape(NBH, 4, P, NCC, 4, P)
        p = p.transpose(0, 5, 1, 4, 2, 3).reshape(NBH, L, L)
        op = np.asarray(r["opart"], f32)
        for bh in range(NBH):
            b, hl = bh // HPC, bh % HPC
            h = HPC * c + hl
            attention[b * H + h] = p[bh]
            context[b, h * P:(h + 1) * P, :] = op[bh * P:(bh + 1) * P, :]
    return context, attention


# revision 29
# speedup vs baseline: 1.6653x; 1.0096x over previous
"""nn_MultiHeadAttention on 8 TRN2 NeuronCores.

IMPORTANT semantics: the reference does a RAW reshape (torch .view style)
  k.reshape(B*H, Lk, d)   with k = [B, L, D] and H*d = D, L = 2048, d = 64.
Since L*D = H * (L*d), "head" i = (b, h) is NOT a feature slice: it is the
contiguous slab = tokens [h*128, (h+1)*128) of batch b, with those 128
tokens' full 1024-dim projected features re-chunked into 2048 virtual
positions m = tok_local*16 + cc, each of 64 dims (cc = feature chunk).

Sharding: token/head-parallel. Core c owns heads {2c, 2c+1} x both batches
= 4 blocks of 128 tokens (bh-major order, bh = b*2 + hl). Everything is
local to the core: projections for its 512 tokens (full D columns),
attention for its 4 heads, and its 512 rows of the output projection.
Host just concatenates token slices (and unscrambles the probs layout).

Device layout per core:
  xT      [1025, 512]  x^T slice + ones row (projection bias via matmul)
  qn/kn   8 tiles [128(col), 512(tok)] f32r  - transposed projections
  qs      8 tiles - qn with chunks shifted one position (SBUF->SBUF DMA)
           so any (cck, ccq) parity pair can share array row groups
  vn -> vaug per bh: [128(tok), 16*65] bf16: per chunk 64 v-cols + ones col
  scores^T blocks [tk=128, tq=128] via K=64 matmuls, 4 per PSUM bank
  softmax denominator = ones-column row of the PV matmul (row 64 of PSUM)
  probs written as [bh, g, cck, tk, (ccq_l, tq)] - host permutes
  o-proj: ctx^T tiles [128(col), 128(tok)] (placed via SBUF->SBUF DMA)
          @ Wo tiles, bias row via ones matmul. opart [512, 1024] local.
"""

import sys

_REPO = "/opt/trn_rl_repo"
if _REPO not in sys.path:
    sys.path.insert(0, _REPO)

import numpy as np
import ml_dtypes

# ---------------------------------------------------------------- config
EXPS_BF16 = True      # exp(scores)/probs-out/v_aug in bf16 (else f32r)
IN_BF16 = True       # activations/weights bf16 (else f32 + f32r matmuls)

B, L, D = 2, 2048, 1024
H, DH = 16, 64
HPC = 2                       # heads per core
N_CORES = 8
TOKC = 512                    # tokens per core (4 bh-blocks of 128)
NBH = B * HPC                 # 4 local (batch, head) units
P = 128
FT = 8                        # full 128-row feature tiles (+1 bias row)
NCC = 16                      # feature chunks of 64 (virtual positions)

_prog = {}


def _build():
    import concourse.bass as bass
    import concourse.bacc as bacc
    import concourse.mybir as mybir
    import concourse.tile as tile

    F32 = mybir.dt.float32
    F32R = mybir.dt.float32r
    BF16 = mybir.dt.bfloat16
    Exp = mybir.ActivationFunctionType.Exp
    MULT = mybir.AluOpType.mult
    PSUM = bass.MemorySpace.PSUM

    IN_DT = BF16 if IN_BF16 else F32R
    EX_DT = BF16 if EXPS_BF16 else F32R

    nc = bacc.Bacc("TRN2", target_bir_lowering=False, debug=False,
                   num_devices=N_CORES)

    xq = nc.declare_dram_parameter("xq", [D + 1, TOKC], IN_DT, isOutput=False)
    xk = nc.declare_dram_parameter("xk", [D + 1, TOKC], IN_DT, isOutput=False)
    xv = nc.declare_dram_parameter("xv", [D + 1, TOKC], IN_DT, isOutput=False)
    wq = nc.declare_dram_parameter("wq", [D + 1, D], IN_DT, isOutput=False)
    wk = nc.declare_dram_parameter("wk", [D + 1, D], IN_DT, isOutput=False)
    wv = nc.declare_dram_parameter("wv", [D + 1, D], IN_DT, isOutput=False)
    wo = nc.declare_dram_parameter("wo", [D + 1, D], IN_DT, isOutput=False)
    probs = nc.declare_dram_parameter("probs", [NBH, 4, P, NCC * 512], EX_DT,
                                      isOutput=True)
    opart = nc.declare_dram_parameter("opart", [TOKC, D], F32, isOutput=True)

    with tile.TileContext(nc) as tc:
        with (
            tc.tile_pool(name="qkst", bufs=1) as qkst,
            tc.tile_pool(name="vaugp", bufs=1) as vaugp,
            tc.tile_pool(name="ctxtp", bufs=1) as ctxtp,
            tc.tile_pool(name="wop", bufs=1) as wop,
            tc.tile_pool(name="onesp", bufs=1) as onesp,
        ):
            kn = [qkst.tile([P, TOKC], IN_DT, tag=f"kn{j}", name=f"kn{j}")
                  for j in range(8)]
            # per-bh q^T in head-sequence order [dd, mq=(ccq,tq)],
            # duplicated to both partition halves so it can pair with
            # either row-group parity of the k chunks
            q2 = [qkst.tile([P, L], IN_DT, tag=f"q2{bh}", name=f"q2{bh}")
                  for bh in range(NBH)]
            vaug = [vaugp.tile([P, NCC * (DH + 1)], EX_DT, tag=f"va{bh}",
                               name=f"va{bh}") for bh in range(NBH)]
            ctxT = [ctxtp.tile([P, 8 * P], IN_DT, tag=f"cx{bh}",
                               name=f"cx{bh}") for bh in range(NBH)]
            wo_t = [[wop.tile([P, 512], IN_DT, tag=f"wo{j}_{oc}",
                              name=f"wo{j}_{oc}") for oc in range(2)]
                    for j in range(8)]
            wo_b = wop.tile([1, D], IN_DT, tag="wo_b")
            ones_f = onesp.tile([1, P], F32, tag="ones_f")
            nc.gpsimd.memset(ones_f[:], 1.0)
            ones1 = onesp.tile([1, P], IN_DT, tag="ones1")
            nc.vector.tensor_copy(ones1[:], ones_f[:])

            # ---------------- phase 1: projections
            with (
                tc.tile_pool(name="xsp", bufs=1) as xsp,
                tc.tile_pool(name="wstr", bufs=1) as wstr,
                tc.tile_pool(name="pp1", bufs=6, space=PSUM) as pp1,
            ):
                xt = {}
                xdram = {"q": xq, "k": xk, "v": xv}
                qn = xsp.tile([P, 8 * TOKC], IN_DT, tag="qn", name="qn")

                # q, k in [col, tok] layout; x tiles on the ACT DMA ring,
                # W row-tiles on the SP ring so loads stream in parallel
                for name, dram in (("q", wq), ("k", wk)):
                    wrow = []
                    for ft in range(FT + 1):
                        kp = P if ft < FT else 1
                        t = xsp.tile([kp, TOKC], IN_DT, tag=f"x{name}{ft}",
                                     name=f"x{name}{ft}")
                        nc.scalar.dma_start(t[:],
                                            xdram[name][ft * P:ft * P + kp, :])
                        xt[name, ft] = t
                        wt = wstr.tile([kp, D], IN_DT, tag=f"w{name}{ft}",
                                       name=f"w{name}{ft}")
                        nc.sync.dma_start(wt[:], dram[ft * P:ft * P + kp, :])
                        wrow.append(wt)
                    for ct in range(8):
                        pt = pp1.tile([P, TOKC], F32, tag="pp1", name="pp1")
                        for ft in range(FT + 1):
                            nc.tensor.matmul(
                                pt[:],
                                wrow[ft][:, ct * P:(ct + 1) * P],
                                xt[name, ft][:],
                                start=(ft == 0), stop=(ft == FT))
                        if name == "q":
                            nc.vector.tensor_copy(
                                qn[:, ct * TOKC:(ct + 1) * TOKC], pt[:])
                        else:
                            nc.vector.tensor_copy(kn[ct][:], pt[:])

                # build q2[bh][{0:64,64:128}, ccq*128:+128] = q chunk ccq
                # of this bh's tokens (both halves identical). One strided
                # DMA per (bh, half, parity) gathers all 8 same-parity
                # chunks: src = qn[par*64:+64, ct*512 + bh*128 (+128)],
                # dst free offsets ccq=2*ct+par -> (2*ct+par)*128.
                qnv = qn.rearrange("p (ct t) -> p ct t", t=TOKC)
                for bh in range(NBH):
                    q2v = q2[bh].rearrange("p (ct pr t) -> p ct pr t",
                                           pr=2, t=P)
                    for r in (0, 64):
                        for par in (0, 1):
                            nc.sync.dma_start(
                                q2v[r:r + 64, :, par, :],
                                qnv[par * 64:par * 64 + 64, :,
                                    bh * P:(bh + 1) * P])

                # v in natural [tok, col] layout, straight into vaug (+ones)
                wvrow = []
                for ft in range(FT + 1):
                    kp = P if ft < FT else 1
                    t = xsp.tile([kp, TOKC], IN_DT, tag=f"xv{ft}",
                                 name=f"xv{ft}")
                    nc.scalar.dma_start(t[:], xv[ft * P:ft * P + kp, :])
                    xt["v", ft] = t
                    wt = wstr.tile([kp, D], IN_DT, tag=f"wv{ft}",
                                   name=f"wv{ft}")
                    nc.sync.dma_start(wt[:], wv[ft * P:ft * P + kp, :])
                    wvrow.append(wt)
                for bh in range(NBH):
                    ts = slice(bh * P, (bh + 1) * P)
                    vview = vaug[bh].rearrange("p (c e) -> p c e", e=DH + 1)
                    for oc in range(2):
                        pt = pp1.tile([P, 512], F32, tag="pp1", name="pp1v")
                        for ft in range(FT + 1):
                            nc.tensor.matmul(pt[:], xt["v", ft][:, ts],
                                             wvrow[ft][:, oc * 512:
                                                       oc * 512 + 512],
                                             start=(ft == 0),
                                             stop=(ft == FT))
                        pview = pt.rearrange("p (c e) -> p c e", e=DH)
                        nc.vector.tensor_copy(
                            vview[:, oc * 8:(oc + 1) * 8, 0:DH], pview[:])
                    nc.gpsimd.memset(vview[:, :, DH:DH + 1], 1.0)

            # ---------------- phase 2: attention
            with (
                tc.tile_pool(name="exps", bufs=6) as exps,
                tc.tile_pool(name="bcp", bufs=4) as bcp,
                tc.tile_pool(name="rzp", bufs=4) as rzp,
                tc.tile_pool(name="ctmp", bufs=4) as ctmp,
                tc.tile_pool(name="osb", bufs=4) as osb,
                tc.tile_pool(name="pps", bufs=2, space=PSUM) as pps,
                tc.tile_pool(name="ppav", bufs=3, space=PSUM) as ppav,
                tc.tile_pool(name="ppo", bufs=1, space=PSUM) as ppo,
            ):
                for j in range(8):
                    for oc in range(2):
                        nc.sync.dma_start(
                            wo_t[j][oc][:],
                            wo[j * P:(j + 1) * P, oc * 512:oc * 512 + 512])
                nc.sync.dma_start(wo_b[:], wo[D:D + 1, :])
                for bh in range(NBH):
                    ts = slice(bh * P, (bh + 1) * P)
                    for g in range(4):
                        pav = ppav.tile([DH + 1, 512], F32, tag="pav",
                                        name="pav")
                        eb = exps.tile([P, NCC * 512], EX_DT, tag="exps",
                                       name="exps")
                        for t in range(8):          # cck pairs 2t, 2t+1
                            pss = pps.tile([P, 1024], F32, tag="pss",
                                           name="pss")
                            for par in (0, 1):
                                cck = 2 * t + par
                                rk = par * 64
                                nc.tensor.matmul(
                                    pss[:, par * 512:par * 512 + 512],
                                    kn[cck // 2][rk:rk + 64, ts],
                                    q2[bh][rk:rk + 64,
                                           g * 512:g * 512 + 512])
                            nc.scalar.activation(
                                eb[:, t * 1024:(t + 1) * 1024], pss[:],
                                Exp, scale=0.125)
                            for par in (0, 1):
                                cck = 2 * t + par
                                nc.tensor.matmul(
                                    pav[:],
                                    vaug[bh][:, cck * (DH + 1):
                                             (cck + 1) * (DH + 1)],
                                    eb[:, cck * 512:(cck + 1) * 512],
                                    start=(cck == 0), stop=(cck == 15))
                        # softmax denominators are row 64 of pav; spread
                        # the 512 Z values across all partitions so the
                        # reciprocal runs 128 lanes wide (a [1,512] slice
                        # would be single-lane and ~3.4us)
                        zrow = rzp.tile([P, 512], F32, tag="zrow",
                                        name="zrow")
                        nc.vector.tensor_copy(zrow[64:65, :], pav[64:65, :])
                        zc = rzp.tile([P, 4], F32, tag="zc", name="zc")
                        nc.sync.dma_start(zc[:], zrow[64:65, :])
                        zr = rzp.tile([P, 4], F32, tag="zr", name="zr")
                        nc.vector.reciprocal_approx_fast(zr[:], zc[:])
                        zrb = rzp.tile([P, 4], EX_DT, tag="zrb", name="zrb")
                        nc.vector.tensor_copy(zrb[:], zr[:])
                        rz0 = rzp.tile([1, 512], EX_DT, tag="rz0",
                                       name="rz0")
                        nc.sync.dma_start(rz0[:], zrb[:])
                        bc = bcp.tile([P, 512], EX_DT, tag="bc", name="bc")
                        nc.gpsimd.partition_broadcast(bc[:], rz0[:])
                        ebv = eb.rearrange("p (q f) -> p q f", f=512)
                        bcb = bc[:].rearrange("p (o f) -> p o f",
                                              o=1).broadcast_to((P, 4, 512))
                        for t4 in range(4):
                            esl = ebv[:, 4 * t4:4 * t4 + 4, :]
                            nc.vector.tensor_tensor(esl, esl, bcb, MULT)
                        nc.sync.dma_start(probs[bh, g], eb[:])
                        ct_ = ctmp.tile([DH, 512], IN_DT, tag="ctmp",
                                        name="ctmp")
                        nc.vector.tensor_tensor(ct_[:], pav[0:DH, :],
                                                bc[0:DH, :], MULT)
                        # scatter ct_ chunks into ctxT: ccq=g*4+l ->
                        # tile col-block ccq//2, rows (ccq%2)*64. Group g
                        # covers ct-blocks 2g, 2g+1; parity == l%2.
                        cxv = ctxT[bh].rearrange("p (ct t) -> p ct t", t=P)
                        ctv = ct_.rearrange("p (j pr t) -> p j pr t",
                                            pr=2, t=P)
                        for par in (0, 1):
                            nc.sync.dma_start(
                                cxv[par * 64:par * 64 + 64,
                                    2 * g:2 * g + 2, :],
                                ctv[:, :, par, :])
                    # ---- output projection for this bh block
                    for oc in range(2):
                        po = ppo.tile([P, 512], F32, tag="po", name="po")
                        for j in range(8):
                            nc.tensor.matmul(po[:],
                                             ctxT[bh][:, j * P:(j + 1) * P],
                                             wo_t[j][oc][:],
                                             start=(j == 0), stop=False)
                        nc.tensor.matmul(po[:], ones1[:],
                                         wo_b[:, oc * 512:oc * 512 + 512],
                                         start=False, stop=True)
                        po_sb = osb.tile([P, 512], F32, tag="po_sb",
                                         name="po_sb")
                        if oc == 0:
                            nc.vector.tensor_copy(po_sb[:], po[:])
                        else:
                            nc.scalar.copy(po_sb[:], po[:])
                        nc.sync.dma_start(
                            opart[bh * P:(bh + 1) * P,
                                  oc * 512:oc * 512 + 512], po_sb[:])

    nc.compile()
    return nc


def _get_prog():
    if "nc" not in _prog:
        _prog["nc"] = _build()
    return _prog["nc"]


def kernel(key, value, query, Wk, bk, Wv, bv, Wq, bq, Wo, bo):
    from concourse.bass_utils import run_bass_kernel_spmd

    f32 = np.float32
    in_np = ml_dtypes.bfloat16 if IN_BF16 else f32

    key, value, query = (np.asarray(t, f32) for t in (key, value, query))
    Wk, bk, Wv, bv, Wq, bq, Wo, bo = (
        np.asarray(t, f32) for t in (Wk, bk, Wv, bv, Wq, bq, Wo, bo))

    xf = {"xq": query.reshape(B * L, D), "xk": key.reshape(B * L, D),
          "xv": value.reshape(B * L, D)}
    w_aug = {"wq": np.concatenate([Wq, bq[None, :]], 0).astype(in_np),
             "wk": np.concatenate([Wk, bk[None, :]], 0).astype(in_np),
             "wv": np.concatenate([Wv, bv[None, :]], 0).astype(in_np),
             "wo": np.concatenate([Wo, bo[None, :]], 0).astype(in_np)}

    ones = np.ones((1, TOKC), f32)
    in_maps = []
    for c in range(N_CORES):
        rows = np.concatenate([
            np.arange((bh // HPC) * L + (HPC * c + bh % HPC) * P,
                      (bh // HPC) * L + (HPC * c + bh % HPC) * P + P)
            for bh in range(NBH)])
        m = dict(w_aug)
        for n in ("xq", "xk", "xv"):
            m[n] = np.ascontiguousarray(
                np.concatenate([xf[n][rows].T, ones], 0)).astype(in_np)
        in_maps.append(m)

    nc = _get_prog()
    res = run_bass_kernel_spmd(nc, in_maps, list(range(N_CORES)),
                               trace=False)

    attention = np.empty((B * H, L, L), f32)
    context = np.empty((B, L, D), f32)
    for c in range(N_CORES):
        r = res.results[c]
        # probs [bh, g, tk, (cck, ccq_l, tq)] -> [bh, mq, mk]
        p = np.asarray(r["probs"]).astype(f32)
        p = p.reshape(NBH, 4, P, NCC, 4, P)
        p = p.transpose(0, 5, 1, 4, 2, 3).reshape(NBH, L, L)
        op = np.asarray(r["opart"], f32)
        for bh in range(NBH):
            b, hl = bh // HPC, bh % HPC
            h = HPC * c + hl
            attention[b * H + h] = p[bh]
            context[b, h * P:(h + 1) * P, :] = op[bh * P:(bh + 1) * P, :]
    return context, attention


# revision 30
# speedup vs baseline: 1.7164x; 1.0307x over previous
"""nn_MultiHeadAttention on 8 TRN2 NeuronCores.

IMPORTANT semantics: the reference does a RAW reshape (torch .view style)
  k.reshape(B*H, Lk, d)   with k = [B, L, D] and H*d = D, L = 2048, d = 64.
Since L*D = H * (L*d), "head" i = (b, h) is NOT a feature slice: it is the
contiguous slab = tokens [h*128, (h+1)*128) of batch b, with those 128
tokens' full 1024-dim projected features re-chunked into 2048 virtual
positions m = tok_local*16 + cc, each of 64 dims (cc = feature chunk).

Sharding: token/head-parallel. Core c owns heads {2c, 2c+1} x both batches
= 4 blocks of 128 tokens (bh-major order, bh = b*2 + hl). Everything is
local to the core: projections for its 512 tokens (full D columns),
attention for its 4 heads, and its 512 rows of the output projection.
Host just concatenates token slices (and unscrambles the probs layout).

Device layout per core:
  xT      [1025, 512]  x^T slice + ones row (projection bias via matmul)
  kn      8 tiles [128(col), 512(tok)] - transposed k projection
  q2      per-bh q^T in head-sequence order [64(dd), 2048(mq)], duplicated
          to both partition halves (SBUF->SBUF DMA) so scores pair with
          either k-chunk row-group parity
  vn -> vaug per bh: [128(tok), 16*65] bf16: per chunk 64 v-cols + ones col
  scores^T blocks [tk=128, tq=128] via K=64 matmuls, 4 per PSUM bank
  softmax denominator = ones-column row of the PV matmul (row 64 of PSUM)
  probs written as [bh, g, tk, (cck, ccq_l, tq)] - host permutes
  o-proj: ctx^T tiles [128(col), 128(tok)] (placed via SBUF->SBUF DMA)
          @ Wo tiles, bias row via ones matmul. opart [512, 1024] local.
"""

import sys

_REPO = "/opt/trn_rl_repo"
if _REPO not in sys.path:
    sys.path.insert(0, _REPO)

import numpy as np
import ml_dtypes

# ---------------------------------------------------------------- config
EXPS_BF16 = True      # exp(scores)/probs-out/v_aug in bf16 (else f32r)
IN_BF16 = True       # activations/weights bf16 (else f32 + f32r matmuls)

B, L, D = 2, 2048, 1024
H, DH = 16, 64
HPC = 2                       # heads per core
N_CORES = 8
TOKC = 512                    # tokens per core (4 bh-blocks of 128)
NBH = B * HPC                 # 4 local (batch, head) units
P = 128
FT = 8                        # full 128-row feature tiles (+1 bias row)
NCC = 16                      # feature chunks of 64 (virtual positions)

_prog = {}


def _build():
    import concourse.bass as bass
    import concourse.bacc as bacc
    import concourse.mybir as mybir
    import concourse.tile as tile

    F32 = mybir.dt.float32
    F32R = mybir.dt.float32r
    BF16 = mybir.dt.bfloat16
    Exp = mybir.ActivationFunctionType.Exp
    MULT = mybir.AluOpType.mult
    PSUM = bass.MemorySpace.PSUM

    IN_DT = BF16 if IN_BF16 else F32R
    EX_DT = BF16 if EXPS_BF16 else F32R

    nc = bacc.Bacc("TRN2", target_bir_lowering=False, debug=False,
                   num_devices=N_CORES)

    xq = nc.declare_dram_parameter("xq", [D + 1, TOKC], IN_DT, isOutput=False)
    xk = nc.declare_dram_parameter("xk", [D + 1, TOKC], IN_DT, isOutput=False)
    xv = nc.declare_dram_parameter("xv", [D + 1, TOKC], IN_DT, isOutput=False)
    wq = nc.declare_dram_parameter("wq", [D + 1, D], IN_DT, isOutput=False)
    wk = nc.declare_dram_parameter("wk", [D + 1, D], IN_DT, isOutput=False)
    wv = nc.declare_dram_parameter("wv", [D + 1, D], IN_DT, isOutput=False)
    wo = nc.declare_dram_parameter("wo", [D + 1, D], IN_DT, isOutput=False)
    probs = nc.declare_dram_parameter("probs", [NBH, 4, P, NCC * 512], EX_DT,
                                      isOutput=True)
    opart = nc.declare_dram_parameter("opart", [TOKC, D], F32, isOutput=True)

    with tile.TileContext(nc) as tc:
        with (
            tc.tile_pool(name="qkst", bufs=1) as qkst,
            tc.tile_pool(name="vaugp", bufs=1) as vaugp,
            tc.tile_pool(name="ctxtp", bufs=1) as ctxtp,
            tc.tile_pool(name="wop", bufs=1) as wop,
            tc.tile_pool(name="onesp", bufs=1) as onesp,
        ):
            kn = [qkst.tile([P, TOKC], IN_DT, tag=f"kn{j}", name=f"kn{j}")
                  for j in range(8)]
            # per-bh q^T in head-sequence order [dd, mq=(ccq,tq)],
            # duplicated to both partition halves so it can pair with
            # either row-group parity of the k chunks
            q2 = [qkst.tile([P, L], IN_DT, tag=f"q2{bh}", name=f"q2{bh}")
                  for bh in range(NBH)]
            vaug = [vaugp.tile([P, NCC * (DH + 1)], EX_DT, tag=f"va{bh}",
                               name=f"va{bh}") for bh in range(NBH)]
            ctxT = [ctxtp.tile([P, 8 * P], IN_DT, tag=f"cx{bh}",
                               name=f"cx{bh}") for bh in range(NBH)]
            wo_t = [[wop.tile([P, 512], IN_DT, tag=f"wo{j}_{oc}",
                              name=f"wo{j}_{oc}") for oc in range(2)]
                    for j in range(8)]
            wo_b = wop.tile([1, D], IN_DT, tag="wo_b")
            ones_f = onesp.tile([1, P], F32, tag="ones_f")
            nc.gpsimd.memset(ones_f[:], 1.0)
            ones1 = onesp.tile([1, P], IN_DT, tag="ones1")
            nc.vector.tensor_copy(ones1[:], ones_f[:])

            # ---------------- phase 1: projections
            with (
                tc.tile_pool(name="xsp", bufs=1) as xsp,
                tc.tile_pool(name="wstr", bufs=1) as wstr,
                tc.tile_pool(name="pp1", bufs=6, space=PSUM) as pp1,
            ):
                xt = {}
                xdram = {"q": xq, "k": xk, "v": xv}
                qn = xsp.tile([P, 8 * TOKC], IN_DT, tag="qn", name="qn")

                # q, k in [col, tok] layout; x tiles on the ACT DMA ring,
                # W row-tiles on the SP ring so loads stream in parallel
                for name, dram in (("q", wq), ("k", wk)):
                    wrow = []
                    for ft in range(FT + 1):
                        kp = P if ft < FT else 1
                        t = xsp.tile([kp, TOKC], IN_DT, tag=f"x{name}{ft}",
                                     name=f"x{name}{ft}")
                        nc.scalar.dma_start(t[:],
                                            xdram[name][ft * P:ft * P + kp, :])
                        xt[name, ft] = t
                        wt = wstr.tile([kp, D], IN_DT, tag=f"w{name}{ft}",
                                       name=f"w{name}{ft}")
                        nc.sync.dma_start(wt[:], dram[ft * P:ft * P + kp, :])
                        wrow.append(wt)
                    for ct in range(8):
                        pt = pp1.tile([P, TOKC], F32, tag="pp1", name="pp1")
                        for ft in range(FT + 1):
                            nc.tensor.matmul(
                                pt[:],
                                wrow[ft][:, ct * P:(ct + 1) * P],
                                xt[name, ft][:],
                                start=(ft == 0), stop=(ft == FT))
                        if name == "q":
                            nc.vector.tensor_copy(
                                qn[:, ct * TOKC:(ct + 1) * TOKC], pt[:])
                        else:
                            nc.vector.tensor_copy(kn[ct][:], pt[:])

                # build q2[bh][{0:64,64:128}, ccq*128:+128] = q chunk ccq
                # of this bh's tokens (both halves identical). One strided
                # DMA per (bh, half, parity) gathers all 8 same-parity
                # chunks: src = qn[par*64:+64, ct*512 + bh*128 (+128)],
                # dst free offsets ccq=2*ct+par -> (2*ct+par)*128.
                qnv = qn.rearrange("p (ct t) -> p ct t", t=TOKC)
                for bh in range(NBH):
                    q2v = q2[bh].rearrange("p (ct pr t) -> p ct pr t",
                                           pr=2, t=P)
                    for r in (0, 64):
                        for par in (0, 1):
                            nc.sync.dma_start(
                                q2v[r:r + 64, :, par, :],
                                qnv[par * 64:par * 64 + 64, :,
                                    bh * P:(bh + 1) * P])

                # v in natural [tok, col] layout, straight into vaug (+ones)
                wvrow = []
                for ft in range(FT + 1):
                    kp = P if ft < FT else 1
                    t = xsp.tile([kp, TOKC], IN_DT, tag=f"xv{ft}",
                                 name=f"xv{ft}")
                    nc.scalar.dma_start(t[:], xv[ft * P:ft * P + kp, :])
                    xt["v", ft] = t
                    wt = wstr.tile([kp, D], IN_DT, tag=f"wv{ft}",
                                   name=f"wv{ft}")
                    nc.sync.dma_start(wt[:], wv[ft * P:ft * P + kp, :])
                    wvrow.append(wt)
                for bh in range(NBH):
                    ts = slice(bh * P, (bh + 1) * P)
                    vview = vaug[bh].rearrange("p (c e) -> p c e", e=DH + 1)
                    for oc in range(2):
                        pt = pp1.tile([P, 512], F32, tag="pp1", name="pp1v")
                        for ft in range(FT + 1):
                            nc.tensor.matmul(pt[:], xt["v", ft][:, ts],
                                             wvrow[ft][:, oc * 512:
                                                       oc * 512 + 512],
                                             start=(ft == 0),
                                             stop=(ft == FT))
                        pview = pt.rearrange("p (c e) -> p c e", e=DH)
                        nc.vector.tensor_copy(
                            vview[:, oc * 8:(oc + 1) * 8, 0:DH], pview[:])
                    nc.gpsimd.memset(vview[:, :, DH:DH + 1], 1.0)

            # ---------------- phase 2: attention
            with (
                tc.tile_pool(name="exps", bufs=6) as exps,
                tc.tile_pool(name="bcp", bufs=4) as bcp,
                tc.tile_pool(name="rzp", bufs=4) as rzp,
                tc.tile_pool(name="ctmp", bufs=4) as ctmp,
                tc.tile_pool(name="osb", bufs=4) as osb,
                tc.tile_pool(name="pps", bufs=2, space=PSUM) as pps,
                tc.tile_pool(name="ppav", bufs=3, space=PSUM) as ppav,
                tc.tile_pool(name="ppo", bufs=1, space=PSUM) as ppo,
            ):
                for j in range(8):
                    for oc in range(2):
                        nc.sync.dma_start(
                            wo_t[j][oc][:],
                            wo[j * P:(j + 1) * P, oc * 512:oc * 512 + 512])
                nc.sync.dma_start(wo_b[:], wo[D:D + 1, :])
                for bh in range(NBH):
                    ts = slice(bh * P, (bh + 1) * P)
                    for g in range(4):
                        pav = ppav.tile([DH + 1, 512], F32, tag="pav",
                                        name="pav")
                        eb = exps.tile([P, NCC * 512], EX_DT, tag="exps",
                                       name="exps")
                        for t in range(8):          # cck pairs 2t, 2t+1
                            pss = pps.tile([P, 1024], F32, tag="pss",
                                           name="pss")
                            for par in (0, 1):
                                cck = 2 * t + par
                                rk = par * 64
                                nc.tensor.matmul(
                                    pss[:, par * 512:par * 512 + 512],
                                    kn[cck // 2][rk:rk + 64, ts],
                                    q2[bh][rk:rk + 64,
                                           g * 512:g * 512 + 512])
                            nc.scalar.activation(
                                eb[:, t * 1024:(t + 1) * 1024], pss[:],
                                Exp, scale=0.125)
                            for par in (0, 1):
                                cck = 2 * t + par
                                nc.tensor.matmul(
                                    pav[:],
                                    vaug[bh][:, cck * (DH + 1):
                                             (cck + 1) * (DH + 1)],
                                    eb[:, cck * 512:(cck + 1) * 512],
                                    start=(cck == 0), stop=(cck == 15))
                        # softmax denominators are row 64 of pav; spread
                        # the 512 Z values across all partitions so the
                        # reciprocal runs 128 lanes wide (a [1,512] slice
                        # would be single-lane and ~3.4us)
                        zrow = rzp.tile([P, 512], F32, tag="zrow",
                                        name="zrow")
                        nc.vector.tensor_copy(zrow[64:65, :], pav[64:65, :])
                        zc = rzp.tile([P, 4], F32, tag="zc", name="zc")
                        nc.sync.dma_start(zc[:], zrow[64:65, :])
                        zr = rzp.tile([P, 4], F32, tag="zr", name="zr")
                        nc.vector.reciprocal_approx_fast(zr[:], zc[:])
                        zrb = rzp.tile([P, 4], EX_DT, tag="zrb", name="zrb")
                        nc.vector.tensor_copy(zrb[:], zr[:])
                        rz0 = rzp.tile([1, 512], EX_DT, tag="rz0",
                                       name="rz0")
                        nc.sync.dma_start(rz0[:], zrb[:])
                        bc = bcp.tile([P, 512], EX_DT, tag="bc", name="bc")
                        nc.gpsimd.partition_broadcast(bc[:], rz0[:])
                        ebv = eb.rearrange("p (q f) -> p q f", f=512)
                        bcb = bc[:].rearrange("p (o f) -> p o f",
                                              o=1).broadcast_to((P, 4, 512))
                        for t4 in range(4):
                            esl = ebv[:, 4 * t4:4 * t4 + 4, :]
                            nc.vector.tensor_tensor(esl, esl, bcb, MULT)
                        nc.sync.dma_start(probs[bh, g], eb[:])
                        ct_ = ctmp.tile([DH, 512], IN_DT, tag="ctmp",
                                        name="ctmp")
                        nc.vector.tensor_tensor(ct_[:], pav[0:DH, :],
                                                bc[0:DH, :], MULT)
                        # scatter ct_ chunks into ctxT: ccq=g*4+l ->
                        # tile col-block ccq//2, rows (ccq%2)*64. Group g
                        # covers ct-blocks 2g, 2g+1; parity == l%2.
                        cxv = ctxT[bh].rearrange("p (ct t) -> p ct t", t=P)
                        ctv = ct_.rearrange("p (j pr t) -> p j pr t",
                                            pr=2, t=P)
                        for par in (0, 1):
                            nc.sync.dma_start(
                                cxv[par * 64:par * 64 + 64,
                                    2 * g:2 * g + 2, :],
                                ctv[:, :, par, :])
                    # ---- output projection for this bh block
                    for oc in range(2):
                        po = ppo.tile([P, 512], F32, tag="po", name="po")
                        for j in range(8):
                            nc.tensor.matmul(po[:],
                                             ctxT[bh][:, j * P:(j + 1) * P],
                                             wo_t[j][oc][:],
                                             start=(j == 0), stop=False)
                        nc.tensor.matmul(po[:], ones1[:],
                                         wo_b[:, oc * 512:oc * 512 + 512],
                                         start=False, stop=True)
                        po_sb = osb.tile([P, 512], F32, tag="po_sb",
                                         name="po_sb")
                        if oc == 0:
                            nc.vector.tensor_copy(po_sb[:], po[:])
                        else:
                            nc.scalar.copy(po_sb[:], po[:])
                        nc.sync.dma_start(
                            opart[bh * P:(bh + 1) * P,
                                  oc * 512:oc * 512 + 512], po_sb[:])

    nc.compile()
    return nc


def _get_prog():
    if "nc" not in _prog:
        _prog["nc"] = _build()
    return _prog["nc"]


def kernel(key, value, query, Wk, bk, Wv, bv, Wq, bq, Wo, bo):
    from concourse.bass_utils import run_bass_kernel_spmd

    f32 = np.float32
    in_np = ml_dtypes.bfloat16 if IN_BF16 else f32

    key, value, query = (np.asarray(t, f32) for t in (key, value, query))
    Wk, bk, Wv, bv, Wq, bq, Wo, bo = (
        np.asarray(t, f32) for t in (Wk, bk, Wv, bv, Wq, bq, Wo, bo))

    xf = {"xq": query.reshape(B * L, D), "xk": key.reshape(B * L, D),
          "xv": value.reshape(B * L, D)}
    w_aug = {"wq": np.concatenate([Wq, bq[None, :]], 0).astype(in_np),
             "wk": np.concatenate([Wk, bk[None, :]], 0).astype(in_np),
             "wv": np.concatenate([Wv, bv[None, :]], 0).astype(in_np),
             "wo": np.concatenate([Wo, bo[None, :]], 0).astype(in_np)}

    ones = np.ones((1, TOKC), f32)
    in_maps = []
    for c in range(N_CORES):
        rows = np.concatenate([
            np.arange((bh // HPC) * L + (HPC * c + bh % HPC) * P,
                      (bh // HPC) * L + (HPC * c + bh % HPC) * P + P)
            for bh in range(NBH)])
        m = dict(w_aug)
        for n in ("xq", "xk", "xv"):
            m[n] = np.ascontiguousarray(
                np.concatenate([xf[n][rows].T, ones], 0)).astype(in_np)
        in_maps.append(m)

    nc = _get_prog()
    res = run_bass_kernel_spmd(nc, in_maps, list(range(N_CORES)),
                               trace=False)

    attention = np.empty((B * H, L, L), f32)
    context = np.empty((B, L, D), f32)
    for c in range(N_CORES):
        r = res.results[c]
        # probs [bh, g, tk, (cck, ccq_l, tq)] -> [bh, mq, mk]
        p = np.asarray(r["probs"]).astype(f32)
        p = p.reshape(NBH, 4, P, NCC, 4, P)
        p = p.transpose(0, 5, 1, 4, 2, 3).reshape(NBH, L, L)
        op = np.asarray(r["opart"], f32)
        for bh in range(NBH):
            b, hl = bh // HPC, bh % HPC
            h = HPC * c + hl
            attention[b * H + h] = p[bh]
            context[b, h * P:(h + 1) * P, :] = op[bh * P:(bh + 1) * P, :]
    return context, attention
